# revision 37
# baseline (speedup 1.0000x reference)
"""Trainium2 Bass kernel for EnhancedGNN (2-layer GCN + all-pairs edge MLP).

Math (N=2048 nodes, F=128 in-features, H=16 hidden):
  h        = relu(Ahat @ (x @ W1) + b1)            [N, 16]
  node_out = Ahat @ (h @ W2) + b2                  [N, 2]
  E[i,j]   = sigmoid(relu([h_i, h_j] @ Wc1 + bc1) @ Wc2 + bc2)   [N, N]
  full_edge_index = all-pairs (row-major)          [2, N^2]
with Ahat = D^-1/2 (A + I) D^-1/2 built from edge_index (in-degree by dst).

Key algebra: [h_i, h_j] @ Wc1 = (h @ Wc1_top)[i] + (h @ Wc1_bot)[j], so the
N^2 x 32 pair-feature matmul collapses to an outer sum of two [N, 16] tables:
  E[i,j] = sigmoid( sum_k Wc2[k] * relu(A'[i,k] + B[j,k]) + bc2 )
  A' = h @ Wc1_top + bc1,  B = h @ Wc1_bot.

Device strategy per core (rows of E sharded, 256 rows/core):
  * conv1 replicated (every core needs the full node table for the B side),
    streamed in 4 column-chunks; all matmul operands fp16, fp32 PSUM.
  * Edge MLP on a (k4, i32) partition layout: p = k4*32 + i32.  For each
    32-row group and k-quadruple kg: DVE tensor_scalar computes
    R[p,j] = fp16(max(B[j, kg*4+k4] + A'[row(i32), kg*4+k4], 0)) in 4x mode,
    then one PE matmul with the block-diagonal stationary
    Wc2sel4[kg][p, i32'] = Wc2[kg*4+k4] * (i32==i32') accumulates the
    k-contraction straight into PSUM (M=32, legal base partitions 0/32/64;
    rows 96:128 use a second PSUM tile).  Sigmoid runs on ACT out of PSUM.
  * Per-core variation lives entirely in input data (AhatT_own slice);
    the program is identical on all 8 cores.
"""
import sys

import numpy as np

if "/opt/trn_rl_repo" not in sys.path:
    sys.path.insert(0, "/opt/trn_rl_repo")

import concourse.bass as bass
import concourse.tile as tile
from concourse import bacc, mybir
from concourse.bass_utils import run_bass_kernel_spmd

N = 2048
F_IN = 128
H = 16
NCORES = 8
ROWS = N // NCORES          # 256 rows of E per core
NCHUNK = 4
CHUNK = N // NCHUNK         # 512
NCB = N // 128              # 16 contraction blocks
f16 = mybir.dt.float16
f32 = mybir.dt.float32
f8 = mybir.dt.float8e4
CF16 = 2 * H + 512 + 130   # packed f16 consts: W1|Wtopb|Wbot4|Wc2sel4|W2
CF32 = 278                   # packed f32 consts: b1col|b2bc|bc2col

_PROG_CACHE = {}


def _build_program():
    nc = bacc.Bacc("TRN2")

    # AhatT pre-tiled host-side: [chunk, cb, 128, CHUNK] so each (chunk, cb)
    # tile is one contiguous 128KB DRAM read.
    AhatT_d = nc.declare_dram_parameter(
        "AhatT_t", [NCHUNK, 128, NCB, CHUNK], f8, isOutput=False
    )
    Ahown_d = nc.declare_dram_parameter(
        "AhatT_own_t", [128, NCB, ROWS], f8, isOutput=False
    )
    xT_d = nc.declare_dram_parameter("xT", [F_IN, N], f16, isOutput=False)
    cf16_d = nc.declare_dram_parameter("cf16", [128, CF16], f16, isOutput=False)
    cf32_d = nc.declare_dram_parameter("cf32", [128, CF32], f32, isOutput=False)
    dinvrow_d = nc.declare_dram_parameter("dinvrow", [1, N], f32, isOutput=False)
    dinvown_d = nc.declare_dram_parameter("dinvown", [1, ROWS], f32, isOutput=False)
    edge_d = nc.declare_dram_parameter("edge_rows", [ROWS, N], f16, isOutput=True)
    node_d = nc.declare_dram_parameter("node_rows", [ROWS, 2], f32, isOutput=True)

    with tile.TileContext(nc) as tc:
        with (
            tc.tile_pool(name="singles", bufs=1) as singles,
            tc.tile_pool(name="ah", bufs=3) as ahpool,
            tc.tile_pool(name="bbf", bufs=2) as bbfpool,
            tc.tile_pool(name="rt", bufs=4) as rtpool,
            tc.tile_pool(name="eo", bufs=3) as eopool,
            tc.tile_pool(name="ps_small", bufs=2, space="PSUM") as ps_small,
            tc.tile_pool(name="ps_bb", bufs=2, space="PSUM") as ps_bb,
            tc.tile_pool(name="ps_la", bufs=2, space="PSUM") as ps_la,
            tc.tile_pool(name="ps_lb", bufs=2, space="PSUM") as ps_lb,
        ):
            # ---- input DMAs --------------------------------------------
            xT_sb = singles.tile([F_IN, N], f16)
            nc.sync.dma_start(out=xT_sb, in_=xT_d[:])
            cf16_sb = singles.tile([128, CF16], f16)
            nc.sync.dma_start(out=cf16_sb, in_=cf16_d[:])
            cf32_sb = singles.tile([128, CF32], f32)
            nc.sync.dma_start(out=cf32_sb, in_=cf32_d[:])
            dinvrow_sb = singles.tile([H, N], f32)
            _dr = dinvrow_d[:]
            nc.gpsimd.dma_start(
                out=dinvrow_sb,
                in_=bass.AP(
                    tensor=_dr.tensor,
                    offset=_dr.offset,
                    ap=[[0, H], [1, N]],
                ),
            )
            dinvown_sb = singles.tile([H, ROWS], f32)
            _do = dinvown_d[:]
            nc.gpsimd.dma_start(
                out=dinvown_sb,
                in_=bass.AP(
                    tensor=_do.tensor,
                    offset=_do.offset,
                    ap=[[0, H], [1, ROWS]],
                ),
            )
            W1_sb = cf16_sb[:, 0:H]
            Wtopb_sb = cf16_sb[0 : H + 1, H : 2 * H]
            Wbot4_sb = cf16_sb[0:H, 2 * H : 2 * H + 512].rearrange(
                "p (g n) -> p g n", g=4
            )
            Wc2sel4_sb = cf16_sb[:, 2 * H + 512 : 2 * H + 512 + 128].rearrange(
                "p (g n) -> p g n", g=4
            )
            W2_sb = cf16_sb[0:H, 2 * H + 512 + 128 : 2 * H + 512 + 130]
            b1col_sb = cf32_sb[0:H, 0:1]
            b2bc_sb = cf32_sb[:, 1:3]
            bc2col_sb = cf32_sb[:, 3:4]
            dinvcols_sb = cf32_sb[:, 4:20]
            dinvocol_sb = cf32_sb[:, 20:22]
            dinvexp_sb = cf32_sb[:, 22:278]  # [p, cb*16+k] = dinv[cb*128+p]

            # Ahat^T own columns, packed [c_local(128), (cb, i_own)]
            ahown_sb = singles.tile([128, NCB, ROWS], f8)
            nc.scalar.dma_start(out=ahown_sb, in_=Ahown_d[:])

            # ---- xw = x @ W1, packed [c_local, (cb, k)] -----------------
            ps_xw = ps_small.tile([128, NCB * H], f32, tag="small")
            for cb in range(NCB):
                nc.tensor.matmul(
                    ps_xw[:, cb * H : (cb + 1) * H],
                    xT_sb[:, cb * 128 : (cb + 1) * 128],
                    W1_sb,
                    start=True,
                    stop=True,
                )
            xw_sb = singles.tile([128, NCB * H], f16)
            nc.vector.tensor_tensor(
                out=xw_sb, in0=ps_xw, in1=dinvexp_sb, op=mybir.AluOpType.mult
            )

            # ---- conv1 on own columns -> hT_own [17, 256] ---------------
            ps_hown = ps_small.tile([H, ROWS], f32, tag="small")
            for cb in range(NCB):
                nc.tensor.matmul(
                    ps_hown,
                    xw_sb[:, cb * H : (cb + 1) * H],
                    ahown_sb[:, cb, :],
                    start=(cb == 0),
                    stop=(cb == NCB - 1),
                )
            hTown_sb = singles.tile([H + 1, ROWS], f16)
            nc.vector.memset(hTown_sb, 1.0)  # row 16 stays all-ones (bias row)
            hraw_own = singles.tile([H, ROWS], f32)
            nc.vector.tensor_tensor(
                out=hraw_own,
                in0=ps_hown,
                in1=dinvown_sb,
                op=mybir.AluOpType.mult,
            )
            nc.scalar.activation(
                out=hTown_sb[0:H, :],
                in_=hraw_own,
                func=mybir.ActivationFunctionType.Relu,
                bias=b1col_sb,
                scale=1.0,
            )

            # ---- A'_own[i_local, k] per row-block, then rearrange -------
            # A3[:, rb*16+k] = A'_rb ; A4[p, k4*8+rb*4+kg] = A3[p, rb*16+kg*4+k4]
            # T4[k4*32+i32, g32*8+rb*4+kg] = A4[g32*32+i32, k4*8+rb*4+kg]
            A3 = singles.tile([128, 32], f32)
            for rb in range(2):
                ps_ap = ps_small.tile([128, H], f32, tag="small")
                nc.tensor.matmul(
                    ps_ap,
                    hTown_sb[:, rb * 128 : (rb + 1) * 128],
                    Wtopb_sb,
                    start=True,
                    stop=True,
                )
                nc.vector.tensor_copy(A3[:, rb * H : (rb + 1) * H], ps_ap)
            A4 = singles.tile([128, 32], f32)
            a3perm = bass.AP(
                tensor=A3.tensor,
                offset=A3.offset,
                ap=[[A3.ap[0][0], 128], [1, 4], [16, 2], [4, 4]],
            )
            nc.vector.tensor_copy(A4, a3perm)
            T4_0 = singles.tile([128, 8], f32)
            T4_1 = singles.tile([128, 8], f32)
            T4_2 = singles.tile([128, 8], f32)
            T4_3 = singles.tile([128, 8], f32)
            T4g = [T4_0, T4_1, T4_2, T4_3]
            astr = A4.ap[0][0]
            for g32 in range(4):
                for k4 in range(4):
                    srcap = bass.AP(
                        tensor=A4.tensor,
                        offset=A4.offset + g32 * 32 * astr + k4 * 8,
                        ap=[[astr, 32], [1, 8]],
                    )
                    nc.gpsimd.dma_start(
                        out=T4g[g32][k4 * 32 : (k4 + 1) * 32, :],
                        in_=srcap,
                    )

            # ---- conv1 full (chunked) + B tables + edge MLP -------------
            # Asymmetric superchunks [512, 1024, 512]: the first edge block
            # depends on only one conv1 chunk (starts ~6us earlier), the
            # middle keeps the 1024-wide TS instruction savings, the last
            # shortens the tail.
            hT_sb = singles.tile([H, N], f16)
            for base, W in ((0, CHUNK), (CHUNK, 2 * CHUNK), (3 * CHUNK, CHUNK)):
                nhalf = W // CHUNK
                bb4_t = bbfpool.tile([128, 4, W], f16, tag="bbf")
                for half in range(nhalf):
                    cs = base + half * CHUNK
                    ch = cs // CHUNK
                    ah_t = ahpool.tile([128, NCB, CHUNK], f8, tag="ah")
                    nc.sync.dma_start(out=ah_t, in_=AhatT_d[ch])
                    ps_h = ps_small.tile([H, CHUNK], f32, tag="small")
                    for cb in range(NCB):
                        nc.tensor.matmul(
                            ps_h,
                            xw_sb[:, cb * H : (cb + 1) * H],
                            ah_t[:, cb, :],
                            start=(cb == 0),
                            stop=(cb == NCB - 1),
                        )
                    hraw_t = eopool.tile([H, CHUNK], f32, tag="hraw")
                    nc.vector.tensor_tensor(
                        out=hraw_t,
                        in0=ps_h,
                        in1=dinvrow_sb[:, cs : cs + CHUNK],
                        op=mybir.AluOpType.mult,
                    )
                    nc.scalar.activation(
                        out=hT_sb[:, cs : cs + CHUNK],
                        in_=hraw_t,
                        func=mybir.ActivationFunctionType.Relu,
                        bias=b1col_sb,
                        scale=1.0,
                    )
                    for kg in range(4):
                        ps_b = ps_bb.tile([128, CHUNK], f32, tag="bb")
                        nc.tensor.matmul(
                            ps_b,
                            Wbot4_sb[:, kg, :],
                            hT_sb[:, cs : cs + CHUNK],
                            start=True,
                            stop=True,
                        )
                        nc.scalar.copy(
                            bb4_t[:, kg, half * CHUNK : (half + 1) * CHUNK], ps_b
                        )

                for rb in range(2):
                    ps_a = []
                    ps_b2 = []
                    for half in range(nhalf):
                        pa = ps_la.tile([96, CHUNK], f32, tag="la", name=f"la{half}")
                        pb = ps_lb.tile([32, CHUNK], f32, tag="lb", name=f"lb{half}")
                        ps_a.append(pa)
                        ps_b2.append(pb)
                    for g32 in range(4):
                        for kg in range(4):
                            r_t = rtpool.tile([128, W], f16, tag="r")
                            nc.vector.tensor_scalar(
                                out=r_t,
                                in0=bb4_t[:, kg, :],
                                scalar1=T4g[g32][
                                    :,
                                    rb * 4 + kg : rb * 4 + kg + 1,
                                ],
                                scalar2=0.0,
                                op0=mybir.AluOpType.add,
                                op1=mybir.AluOpType.max,
                            )
                            for half in range(nhalf):
                                out_ps = (
                                    ps_a[half][g32 * 32 : (g32 + 1) * 32, :]
                                    if g32 < 3
                                    else ps_b2[half]
                                )
                                nc.tensor.matmul(
                                    out_ps,
                                    Wc2sel4_sb[:, kg, :],
                                    r_t[:, half * CHUNK : (half + 1) * CHUNK],
                                    start=(kg == 0),
                                    stop=(kg == 3),
                                )
                    for half in range(nhalf):
                        cs = base + half * CHUNK
                        e_t = eopool.tile([96, CHUNK], f16, tag="e")
                        nc.scalar.activation(
                            out=e_t,
                            in_=ps_a[half],
                            func=mybir.ActivationFunctionType.Sigmoid,
                            bias=bc2col_sb[0:96, :],
                            scale=1.0,
                        )
                        e_t2 = eopool.tile([32, CHUNK], f16, tag="e2")
                        nc.scalar.activation(
                            out=e_t2,
                            in_=ps_b2[half],
                            func=mybir.ActivationFunctionType.Sigmoid,
                            bias=bc2col_sb[0:32, :],
                            scale=1.0,
                        )
                        nc.scalar.dma_start(
                            out=edge_d[rb * 128 : rb * 128 + 96, cs : cs + CHUNK],
                            in_=e_t,
                        )
                        nc.scalar.dma_start(
                            out=edge_d[rb * 128 + 96 : (rb + 1) * 128, cs : cs + CHUNK],
                            in_=e_t2,
                        )

            # ---- conv2: hw2 = h @ W2 packed [c_local, (cb, o)] ----------
            h2T_sb = singles.tile([H, N], f16)
            nc.vector.tensor_tensor(
                out=h2T_sb,
                in0=hT_sb,
                in1=dinvrow_sb,
                op=mybir.AluOpType.mult,
            )
            ps_hw2 = ps_small.tile([128, NCB * 2], f32, tag="small")
            for cb in range(NCB):
                nc.tensor.matmul(
                    ps_hw2[:, cb * 2 : (cb + 1) * 2],
                    h2T_sb[:, cb * 128 : (cb + 1) * 128],
                    W2_sb,
                    start=True,
                    stop=True,
                )
            hw2_sb = singles.tile([128, NCB * 2], f16)
            nc.vector.tensor_copy(hw2_sb, ps_hw2)

            for rb in range(2):
                ps_no = ps_small.tile([128, 2], f32, tag="small")
                for cb in range(NCB):
                    nc.tensor.matmul(
                        ps_no,
                        ahown_sb[:, cb, rb * 128 : (rb + 1) * 128],
                        hw2_sb[:, cb * 2 : (cb + 1) * 2],
                        start=(cb == 0),
                        stop=(cb == NCB - 1),
                    )
                no_sb = eopool.tile([128, 2], f32, tag="no")
                nc.vector.tensor_scalar(
                    out=no_sb,
                    in0=ps_no,
                    scalar1=dinvocol_sb[:, rb : rb + 1],
                    scalar2=None,
                    op0=mybir.AluOpType.mult,
                )
                nc.vector.tensor_tensor(
                    out=no_sb, in0=no_sb, in1=b2bc_sb, op=mybir.AluOpType.add
                )
                nc.scalar.dma_start(
                    out=node_d[rb * 128 : (rb + 1) * 128, :], in_=no_sb
                )

    nc.finalize()
    return nc


def get_program():
    if "nc" not in _PROG_CACHE:
        _PROG_CACHE["nc"] = _build_program()
    return _PROG_CACHE["nc"]


def _host_prep(x, edge_index, W1, b1, W2, b2, Wc1, bc1, Wc2, bc2):
    x = np.asarray(x, dtype=np.float32)
    ei = np.asarray(edge_index)
    src = ei[0].astype(np.int64)
    dst = ei[1].astype(np.int64)
    W1 = np.asarray(W1, np.float32)
    b1 = np.asarray(b1, np.float32)
    W2 = np.asarray(W2, np.float32)
    b2 = np.asarray(b2, np.float32)
    Wc1 = np.asarray(Wc1, np.float32)
    bc1 = np.asarray(bc1, np.float32)
    Wc2 = np.asarray(Wc2, np.float32)
    bc2 = np.asarray(bc2, np.float32)

    deg = (np.bincount(dst, minlength=N) + 1).astype(np.float32)
    dinv = 1.0 / np.sqrt(deg)
    CT = np.zeros((N, N), np.float32)
    np.add.at(CT, (src, dst), 1.0)
    idx = np.arange(N)
    CT[idx, idx] += 1.0
    assert CT.max() <= 16, "edge multiplicity too high for exact fp8 counts"
    f8np = mybir.dt.np(f8)
    CT8 = CT.astype(f8np)

    xT16 = np.ascontiguousarray(x.T).astype(np.float16)
    W1_16 = W1.astype(np.float16)
    Wtopb = np.concatenate([Wc1[:H], bc1[None, :]], axis=0).astype(np.float16)
    # Wbotrep4[f, kg, k4*32+i32] = Wc1_bot[f, kg*4+k4]
    Wbot = Wc1[H:].astype(np.float16)            # [16, 16]
    Wbotrep4 = np.zeros((H, 4, 128), np.float16)
    for kg in range(4):
        for k4 in range(4):
            Wbotrep4[:, kg, k4 * 32 : (k4 + 1) * 32] = Wbot[:, kg * 4 + k4][:, None]
    # Wc2sel4[k4*32+i32, kg, i32'] = Wc2[kg*4+k4] * (i32 == i32')
    Wc2sel4 = np.zeros((128, 4, 32), np.float16)
    eye32 = np.eye(32, dtype=np.float16)
    for kg in range(4):
        for k4 in range(4):
            Wc2sel4[k4 * 32 : (k4 + 1) * 32, kg, :] = (
                np.float16(Wc2[kg * 4 + k4, 0]) * eye32
            )
    W2_16 = W2.astype(np.float16)
    b1col = b1.reshape(H, 1).astype(np.float32)
    b2bc = np.tile(b2.reshape(1, 2), (128, 1)).astype(np.float32)
    bc2col = np.full((128, 1), bc2[0], np.float32)

    # pre-tile: AhatT_t[ch, p, cb, j] = CT8[cb*128+p, ch*CHUNK+j]
    AhatT_t = np.ascontiguousarray(
        CT8.reshape(NCB, 128, NCHUNK, CHUNK).transpose(2, 1, 0, 3)
    )
    cf16 = np.zeros((128, CF16), np.float16)
    cf16[:, 0:H] = W1_16
    cf16[0 : H + 1, H : 2 * H] = Wtopb
    cf16[0:H, 2 * H : 2 * H + 512] = Wbotrep4.reshape(H, 512)
    cf16[:, 2 * H + 512 : 2 * H + 512 + 128] = Wc2sel4.reshape(128, 128)
    cf16[0:H, 2 * H + 512 + 128 : 2 * H + 512 + 130] = W2_16
    cf32 = np.zeros((128, CF32), np.float32)
    cf32[0:H, 0:1] = b1col
    cf32[:, 1:3] = b2bc
    cf32[:, 3:4] = bc2col
    cf32[:, 4:20] = dinv.reshape(NCB, 128).T  # dinvcols[p, cb] = dinv[cb*128+p]
    cf32[:, 22:278] = np.repeat(dinv.reshape(NCB, 128).T, H, axis=1)
    shared = {
        "AhatT_t": AhatT_t,
        "xT": xT16,
        "cf16": cf16,
        "dinvrow": dinv.reshape(1, N),
    }
    in_maps = []
    for c in range(NCORES):
        m = dict(shared)
        m["AhatT_own_t"] = np.ascontiguousarray(
            CT8[:, c * ROWS : (c + 1) * ROWS]
            .reshape(NCB, 128, ROWS)
            .transpose(1, 0, 2)
        )
        dn = dinv[c * ROWS : (c + 1) * ROWS]
        m["dinvown"] = np.ascontiguousarray(dn.reshape(1, ROWS))
        cfc = cf32.copy()
        cfc[:, 20:22] = dn.reshape(2, 128).T  # dinvocol[p, rb] = dinv[own + rb*128+p]
        m["cf32"] = cfc
        in_maps.append(m)
    return in_maps


def kernel(x, edge_index, W1, b1, W2, b2, Wc1, bc1, Wc2, bc2, _res_out=None):
    in_maps = _host_prep(x, edge_index, W1, b1, W2, b2, Wc1, bc1, Wc2, bc2)
    nc = get_program()
    try:
        res = run_bass_kernel_spmd(nc, in_maps, list(range(NCORES)))
    except Exception:
        # transient device states (e.g. NRT_EXEC_UNIT_UNRECOVERABLE after a
        # wedged prior run) usually clear on retry
        import time as _t

        _t.sleep(2.0)
        res = run_bass_kernel_spmd(nc, in_maps, list(range(NCORES)))
    if _res_out is not None:
        _res_out.append(res)

    node_out = np.concatenate(
        [res.results[c]["node_rows"] for c in range(NCORES)], axis=0
    )
    edge_out = (
        np.concatenate(
            [res.results[c]["edge_rows"] for c in range(NCORES)], axis=0
        )
        .astype(np.float32)
        .reshape(-1)
    )
    ar = np.arange(N, dtype=np.int32)
    full_edge_index = np.stack([np.repeat(ar, N), np.tile(ar, N)])
    return node_out, edge_out, full_edge_index


# revision 38
# speedup vs baseline: 1.0411x; 1.0411x over previous
"""Trainium2 Bass kernel for EnhancedGNN (2-layer GCN + all-pairs edge MLP).

Math (N=2048 nodes, F=128 in-features, H=16 hidden):
  h        = relu(Ahat @ (x @ W1) + b1)            [N, 16]
  node_out = Ahat @ (h @ W2) + b2                  [N, 2]
  E[i,j]   = sigmoid(relu([h_i, h_j] @ Wc1 + bc1) @ Wc2 + bc2)   [N, N]
  full_edge_index = all-pairs (row-major)          [2, N^2]
with Ahat = D^-1/2 (A + I) D^-1/2 built from edge_index (in-degree by dst).

Key algebra: [h_i, h_j] @ Wc1 = (h @ Wc1_top)[i] + (h @ Wc1_bot)[j], so the
N^2 x 32 pair-feature matmul collapses to an outer sum of two [N, 16] tables:
  E[i,j] = sigmoid( sum_k Wc2[k] * relu(A'[i,k] + B[j,k]) + bc2 )
  A' = h @ Wc1_top + bc1,  B = h @ Wc1_bot.

Device strategy per core (rows of E sharded, 256 rows/core):
  * conv1 replicated (every core needs the full node table for the B side),
    streamed in 4 column-chunks; all matmul operands fp16, fp32 PSUM.
  * Edge MLP on a (k4, i32) partition layout: p = k4*32 + i32.  For each
    32-row group and k-quadruple kg: DVE tensor_scalar computes
    R[p,j] = fp16(max(B[j, kg*4+k4] + A'[row(i32), kg*4+k4], 0)) in 4x mode,
    then one PE matmul with the block-diagonal stationary
    Wc2sel4[kg][p, i32'] = Wc2[kg*4+k4] * (i32==i32') accumulates the
    k-contraction straight into PSUM (M=32, legal base partitions 0/32/64;
    rows 96:128 use a second PSUM tile).  Sigmoid runs on ACT out of PSUM.
  * Per-core variation lives entirely in input data (AhatT_own slice);
    the program is identical on all 8 cores.
"""
import sys

import numpy as np

if "/opt/trn_rl_repo" not in sys.path:
    sys.path.insert(0, "/opt/trn_rl_repo")

import concourse.bass as bass
import concourse.tile as tile
from concourse import bacc, mybir
from concourse.bass_utils import run_bass_kernel_spmd

N = 2048
F_IN = 128
H = 16
NCORES = 8
ROWS = N // NCORES          # 256 rows of E per core
NCHUNK = 4
CHUNK = N // NCHUNK         # 512
NCB = N // 128              # 16 contraction blocks
f16 = mybir.dt.float16
f32 = mybir.dt.float32
f8 = mybir.dt.float8e4
CF16 = 2 * H + 512 + 130   # packed f16 consts: W1|Wtopb|Wbot4|Wc2sel4|W2
CF32 = 278                   # packed f32 consts: b1col|b2bc|bc2col

_PROG_CACHE = {}


def _build_program():
    nc = bacc.Bacc("TRN2")

    # AhatT pre-tiled host-side: [chunk, cb, 128, CHUNK] so each (chunk, cb)
    # tile is one contiguous 128KB DRAM read.
    AhatT_d = nc.declare_dram_parameter(
        "AhatT_t", [NCHUNK, 128, NCB, CHUNK], f8, isOutput=False
    )
    Ahown_d = nc.declare_dram_parameter(
        "AhatT_own_t", [128, NCB, ROWS], f8, isOutput=False
    )
    xT_d = nc.declare_dram_parameter("xT", [F_IN, N], f16, isOutput=False)
    cf16_d = nc.declare_dram_parameter("cf16", [128, CF16], f16, isOutput=False)
    cf32_d = nc.declare_dram_parameter("cf32", [128, CF32], f32, isOutput=False)
    dinvrow_d = nc.declare_dram_parameter("dinvrow", [1, N], f32, isOutput=False)
    dinvown_d = nc.declare_dram_parameter("dinvown", [1, ROWS], f32, isOutput=False)
    edge_d = nc.declare_dram_parameter("edge_rows", [ROWS, N], f16, isOutput=True)
    node_d = nc.declare_dram_parameter("node_rows", [ROWS, 2], f32, isOutput=True)

    with tile.TileContext(nc) as tc:
        with (
            tc.tile_pool(name="singles", bufs=1) as singles,
            tc.tile_pool(name="ah", bufs=3) as ahpool,
            tc.tile_pool(name="bbf", bufs=2) as bbfpool,
            tc.tile_pool(name="rt", bufs=4) as rtpool,
            tc.tile_pool(name="eo", bufs=3) as eopool,
            tc.tile_pool(name="ps_small", bufs=2, space="PSUM") as ps_small,
            tc.tile_pool(name="ps_bb", bufs=2, space="PSUM") as ps_bb,
            tc.tile_pool(name="ps_la", bufs=2, space="PSUM") as ps_la,
            tc.tile_pool(name="ps_lb", bufs=2, space="PSUM") as ps_lb,
        ):
            # ---- input DMAs --------------------------------------------
            xT_sb = singles.tile([F_IN, N], f16)
            nc.sync.dma_start(out=xT_sb, in_=xT_d[:])
            cf16_sb = singles.tile([128, CF16], f16)
            nc.sync.dma_start(out=cf16_sb, in_=cf16_d[:])
            cf32_sb = singles.tile([128, CF32], f32)
            nc.sync.dma_start(out=cf32_sb, in_=cf32_d[:])
            dinvrow_sb = singles.tile([H, N], f32)
            _dr = dinvrow_d[:]
            nc.gpsimd.dma_start(
                out=dinvrow_sb,
                in_=bass.AP(
                    tensor=_dr.tensor,
                    offset=_dr.offset,
                    ap=[[0, H], [1, N]],
                ),
            )
            dinvown_sb = singles.tile([H, ROWS], f32)
            _do = dinvown_d[:]
            nc.gpsimd.dma_start(
                out=dinvown_sb,
                in_=bass.AP(
                    tensor=_do.tensor,
                    offset=_do.offset,
                    ap=[[0, H], [1, ROWS]],
                ),
            )
            W1_sb = cf16_sb[:, 0:H]
            Wtopb_sb = cf16_sb[0 : H + 1, H : 2 * H]
            Wbot4_sb = cf16_sb[0:H, 2 * H : 2 * H + 512].rearrange(
                "p (g n) -> p g n", g=4
            )
            Wc2sel4_sb = cf16_sb[:, 2 * H + 512 : 2 * H + 512 + 128].rearrange(
                "p (g n) -> p g n", g=4
            )
            W2_sb = cf16_sb[0:H, 2 * H + 512 + 128 : 2 * H + 512 + 130]
            b1col_sb = cf32_sb[0:H, 0:1]
            b2bc_sb = cf32_sb[:, 1:3]
            bc2col_sb = cf32_sb[:, 3:4]
            dinvcols_sb = cf32_sb[:, 4:20]
            dinvocol_sb = cf32_sb[:, 20:22]
            dinvexp_sb = cf32_sb[:, 22:278]  # [p, cb*16+k] = dinv[cb*128+p]

            # Ahat^T own columns, packed [c_local(128), (cb, i_own)]
            ahown_sb = singles.tile([128, NCB, ROWS], f8)
            nc.scalar.dma_start(out=ahown_sb, in_=Ahown_d[:])

            # ---- xw = x @ W1, packed [c_local, (cb, k)] -----------------
            ps_xw = ps_small.tile([128, NCB * H], f32, tag="small")
            for cb in range(NCB):
                nc.tensor.matmul(
                    ps_xw[:, cb * H : (cb + 1) * H],
                    xT_sb[:, cb * 128 : (cb + 1) * 128],
                    W1_sb,
                    start=True,
                    stop=True,
                )
            xw_sb = singles.tile([128, NCB * H], f16)
            nc.vector.tensor_tensor(
                out=xw_sb, in0=ps_xw, in1=dinvexp_sb, op=mybir.AluOpType.mult
            )

            # ---- conv1 on own columns -> hT_own [17, 256] ---------------
            ps_hown = ps_small.tile([H, ROWS], f32, tag="small")
            for cb in range(NCB):
                nc.tensor.matmul(
                    ps_hown,
                    xw_sb[:, cb * H : (cb + 1) * H],
                    ahown_sb[:, cb, :],
                    start=(cb == 0),
                    stop=(cb == NCB - 1),
                )
            hTown_sb = singles.tile([H + 1, ROWS], f16)
            nc.vector.memset(hTown_sb, 1.0)  # row 16 stays all-ones (bias row)
            hraw_own = singles.tile([H, ROWS], f32)
            nc.vector.tensor_tensor(
                out=hraw_own,
                in0=ps_hown,
                in1=dinvown_sb,
                op=mybir.AluOpType.mult,
            )
            nc.scalar.activation(
                out=hTown_sb[0:H, :],
                in_=hraw_own,
                func=mybir.ActivationFunctionType.Relu,
                bias=b1col_sb,
                scale=1.0,
            )

            # ---- A'_own[i_local, k] per row-block, then rearrange -------
            # A3[:, rb*16+k] = A'_rb ; A4[p, k4*8+rb*4+kg] = A3[p, rb*16+kg*4+k4]
            # T4[k4*32+i32, g32*8+rb*4+kg] = A4[g32*32+i32, k4*8+rb*4+kg]
            A3 = singles.tile([128, 32], f32)
            for rb in range(2):
                ps_ap = ps_small.tile([128, H], f32, tag="small")
                nc.tensor.matmul(
                    ps_ap,
                    hTown_sb[:, rb * 128 : (rb + 1) * 128],
                    Wtopb_sb,
                    start=True,
                    stop=True,
                )
                nc.vector.tensor_copy(A3[:, rb * H : (rb + 1) * H], ps_ap)
            A4 = singles.tile([128, 32], f32)
            a3perm = bass.AP(
                tensor=A3.tensor,
                offset=A3.offset,
                ap=[[A3.ap[0][0], 128], [1, 4], [16, 2], [4, 4]],
            )
            nc.vector.tensor_copy(A4, a3perm)
            T4_0 = singles.tile([128, 8], f32)
            T4_1 = singles.tile([128, 8], f32)
            T4_2 = singles.tile([128, 8], f32)
            T4_3 = singles.tile([128, 8], f32)
            T4g = [T4_0, T4_1, T4_2, T4_3]
            astr = A4.ap[0][0]
            for g32 in range(4):
                for k4 in range(4):
                    srcap = bass.AP(
                        tensor=A4.tensor,
                        offset=A4.offset + g32 * 32 * astr + k4 * 8,
                        ap=[[astr, 32], [1, 8]],
                    )
                    nc.gpsimd.dma_start(
                        out=T4g[g32][k4 * 32 : (k4 + 1) * 32, :],
                        in_=srcap,
                    )

            # ---- conv1 full (chunked) + B tables + edge MLP -------------
            # Superchunks of 1024 columns: TS runs [128, 1024] (one per
            # (g32, kg)), feeding two 512-wide PE matmuls.
            hT_sb = singles.tile([H, N], f16)
            for sc in range(2):
                bb4_t = bbfpool.tile([128, 4, 2 * CHUNK], f16, tag="bbf")
                for half in range(2):
                    ch = sc * 2 + half
                    cs = ch * CHUNK
                    ah_t = ahpool.tile([128, NCB, CHUNK], f8, tag="ah")
                    nc.sync.dma_start(out=ah_t, in_=AhatT_d[ch])
                    ps_h = ps_small.tile([H, CHUNK], f32, tag="small")
                    for cb in range(NCB):
                        nc.tensor.matmul(
                            ps_h,
                            xw_sb[:, cb * H : (cb + 1) * H],
                            ah_t[:, cb, :],
                            start=(cb == 0),
                            stop=(cb == NCB - 1),
                        )
                    hraw_t = eopool.tile([H, CHUNK], f32, tag="hraw")
                    nc.vector.tensor_tensor(
                        out=hraw_t,
                        in0=ps_h,
                        in1=dinvrow_sb[:, cs : cs + CHUNK],
                        op=mybir.AluOpType.mult,
                    )
                    nc.scalar.activation(
                        out=hT_sb[:, cs : cs + CHUNK],
                        in_=hraw_t,
                        func=mybir.ActivationFunctionType.Relu,
                        bias=b1col_sb,
                        scale=1.0,
                    )
                    for kg in range(4):
                        ps_b = ps_bb.tile([128, CHUNK], f32, tag="bb")
                        nc.tensor.matmul(
                            ps_b,
                            Wbot4_sb[:, kg, :],
                            hT_sb[:, cs : cs + CHUNK],
                            start=True,
                            stop=True,
                        )
                        nc.scalar.copy(
                            bb4_t[:, kg, half * CHUNK : (half + 1) * CHUNK], ps_b
                        )

                for rb in range(2):
                    ps_a0 = ps_la.tile([96, CHUNK], f32, tag="la")
                    ps_a1 = ps_la.tile([96, CHUNK], f32, tag="la")
                    ps_b20 = ps_lb.tile([32, CHUNK], f32, tag="lb")
                    ps_b21 = ps_lb.tile([32, CHUNK], f32, tag="lb")
                    ps_a = [ps_a0, ps_a1]
                    ps_b2 = [ps_b20, ps_b21]
                    for g32 in range(4):
                        for kg in range(4):
                            r_t = rtpool.tile([128, 2 * CHUNK], f16, tag="r")
                            nc.vector.tensor_scalar(
                                out=r_t,
                                in0=bb4_t[:, kg, :],
                                scalar1=T4g[g32][
                                    :,
                                    rb * 4 + kg : rb * 4 + kg + 1,
                                ],
                                scalar2=0.0,
                                op0=mybir.AluOpType.add,
                                op1=mybir.AluOpType.max,
                            )
                            for half in range(2):
                                out_ps = (
                                    ps_a[half][g32 * 32 : (g32 + 1) * 32, :]
                                    if g32 < 3
                                    else ps_b2[half]
                                )
                                nc.tensor.matmul(
                                    out_ps,
                                    Wc2sel4_sb[:, kg, :],
                                    r_t[:, half * CHUNK : (half + 1) * CHUNK],
                                    start=(kg == 0),
                                    stop=(kg == 3),
                                )
                    for half in range(2):
                        cs = (sc * 2 + half) * CHUNK
                        e_t = eopool.tile([96, CHUNK], f16, tag="e")
                        nc.scalar.activation(
                            out=e_t,
                            in_=ps_a[half],
                            func=mybir.ActivationFunctionType.Sigmoid,
                            bias=bc2col_sb[0:96, :],
                            scale=1.0,
                        )
                        e_t2 = eopool.tile([32, CHUNK], f16, tag="e2")
                        nc.scalar.activation(
                            out=e_t2,
                            in_=ps_b2[half],
                            func=mybir.ActivationFunctionType.Sigmoid,
                            bias=bc2col_sb[0:32, :],
                            scale=1.0,
                        )
                        nc.scalar.dma_start(
                            out=edge_d[rb * 128 : rb * 128 + 96, cs : cs + CHUNK],
                            in_=e_t,
                        )
                        nc.scalar.dma_start(
                            out=edge_d[rb * 128 + 96 : (rb + 1) * 128, cs : cs + CHUNK],
                            in_=e_t2,
                        )

            # ---- conv2: hw2 = h @ W2 packed [c_local, (cb, o)] ----------
            h2T_sb = singles.tile([H, N], f16)
            nc.vector.tensor_tensor(
                out=h2T_sb,
                in0=hT_sb,
                in1=dinvrow_sb,
                op=mybir.AluOpType.mult,
            )
            ps_hw2 = ps_small.tile([128, NCB * 2], f32, tag="small")
            for cb in range(NCB):
                nc.tensor.matmul(
                    ps_hw2[:, cb * 2 : (cb + 1) * 2],
                    h2T_sb[:, cb * 128 : (cb + 1) * 128],
                    W2_sb,
                    start=True,
                    stop=True,
                )
            hw2_sb = singles.tile([128, NCB * 2], f16)
            nc.vector.tensor_copy(hw2_sb, ps_hw2)

            for rb in range(2):
                ps_no = ps_small.tile([128, 2], f32, tag="small")
                for cb in range(NCB):
                    nc.tensor.matmul(
                        ps_no,
                        ahown_sb[:, cb, rb * 128 : (rb + 1) * 128],
                        hw2_sb[:, cb * 2 : (cb + 1) * 2],
                        start=(cb == 0),
                        stop=(cb == NCB - 1),
                    )
                no_sb = eopool.tile([128, 2], f32, tag="no")
                nc.vector.tensor_scalar(
                    out=no_sb,
                    in0=ps_no,
                    scalar1=dinvocol_sb[:, rb : rb + 1],
                    scalar2=None,
                    op0=mybir.AluOpType.mult,
                )
                nc.vector.tensor_tensor(
                    out=no_sb, in0=no_sb, in1=b2bc_sb, op=mybir.AluOpType.add
                )
                nc.scalar.dma_start(
                    out=node_d[rb * 128 : (rb + 1) * 128, :], in_=no_sb
                )

    nc.finalize()
    return nc


def get_program():
    if "nc" not in _PROG_CACHE:
        _PROG_CACHE["nc"] = _build_program()
    return _PROG_CACHE["nc"]


def _host_prep(x, edge_index, W1, b1, W2, b2, Wc1, bc1, Wc2, bc2):
    x = np.asarray(x, dtype=np.float32)
    ei = np.asarray(edge_index)
    src = ei[0].astype(np.int64)
    dst = ei[1].astype(np.int64)
    W1 = np.asarray(W1, np.float32)
    b1 = np.asarray(b1, np.float32)
    W2 = np.asarray(W2, np.float32)
    b2 = np.asarray(b2, np.float32)
    Wc1 = np.asarray(Wc1, np.float32)
    bc1 = np.asarray(bc1, np.float32)
    Wc2 = np.asarray(Wc2, np.float32)
    bc2 = np.asarray(bc2, np.float32)

    deg = (np.bincount(dst, minlength=N) + 1).astype(np.float32)
    dinv = 1.0 / np.sqrt(deg)
    CT = np.zeros((N, N), np.float32)
    np.add.at(CT, (src, dst), 1.0)
    idx = np.arange(N)
    CT[idx, idx] += 1.0
    assert CT.max() <= 16, "edge multiplicity too high for exact fp8 counts"
    f8np = mybir.dt.np(f8)
    CT8 = CT.astype(f8np)

    xT16 = np.ascontiguousarray(x.T).astype(np.float16)
    W1_16 = W1.astype(np.float16)
    Wtopb = np.concatenate([Wc1[:H], bc1[None, :]], axis=0).astype(np.float16)
    # Wbotrep4[f, kg, k4*32+i32] = Wc1_bot[f, kg*4+k4]
    Wbot = Wc1[H:].astype(np.float16)            # [16, 16]
    Wbotrep4 = np.zeros((H, 4, 128), np.float16)
    for kg in range(4):
        for k4 in range(4):
            Wbotrep4[:, kg, k4 * 32 : (k4 + 1) * 32] = Wbot[:, kg * 4 + k4][:, None]
    # Wc2sel4[k4*32+i32, kg, i32'] = Wc2[kg*4+k4] * (i32 == i32')
    Wc2sel4 = np.zeros((128, 4, 32), np.float16)
    eye32 = np.eye(32, dtype=np.float16)
    for kg in range(4):
        for k4 in range(4):
            Wc2sel4[k4 * 32 : (k4 + 1) * 32, kg, :] = (
                np.float16(Wc2[kg * 4 + k4, 0]) * eye32
            )
    W2_16 = W2.astype(np.float16)
    b1col = b1.reshape(H, 1).astype(np.float32)
    b2bc = np.tile(b2.reshape(1, 2), (128, 1)).astype(np.float32)
    bc2col = np.full((128, 1), bc2[0], np.float32)

    # pre-tile: AhatT_t[ch, p, cb, j] = CT8[cb*128+p, ch*CHUNK+j]
    AhatT_t = np.ascontiguousarray(
        CT8.reshape(NCB, 128, NCHUNK, CHUNK).transpose(2, 1, 0, 3)
    )
    cf16 = np.zeros((128, CF16), np.float16)
    cf16[:, 0:H] = W1_16
    cf16[0 : H + 1, H : 2 * H] = Wtopb
    cf16[0:H, 2 * H : 2 * H + 512] = Wbotrep4.reshape(H, 512)
    cf16[:, 2 * H + 512 : 2 * H + 512 + 128] = Wc2sel4.reshape(128, 128)
    cf16[0:H, 2 * H + 512 + 128 : 2 * H + 512 + 130] = W2_16
    cf32 = np.zeros((128, CF32), np.float32)
    cf32[0:H, 0:1] = b1col
    cf32[:, 1:3] = b2bc
    cf32[:, 3:4] = bc2col
    cf32[:, 4:20] = dinv.reshape(NCB, 128).T  # dinvcols[p, cb] = dinv[cb*128+p]
    cf32[:, 22:278] = np.repeat(dinv.reshape(NCB, 128).T, H, axis=1)
    shared = {
        "AhatT_t": AhatT_t,
        "xT": xT16,
        "cf16": cf16,
        "dinvrow": dinv.reshape(1, N),
    }
    in_maps = []
    for c in range(NCORES):
        m = dict(shared)
        m["AhatT_own_t"] = np.ascontiguousarray(
            CT8[:, c * ROWS : (c + 1) * ROWS]
            .reshape(NCB, 128, ROWS)
            .transpose(1, 0, 2)
        )
        dn = dinv[c * ROWS : (c + 1) * ROWS]
        m["dinvown"] = np.ascontiguousarray(dn.reshape(1, ROWS))
        cfc = cf32.copy()
        cfc[:, 20:22] = dn.reshape(2, 128).T  # dinvocol[p, rb] = dinv[own + rb*128+p]
        m["cf32"] = cfc
        in_maps.append(m)
    return in_maps


def kernel(x, edge_index, W1, b1, W2, b2, Wc1, bc1, Wc2, bc2, _res_out=None):
    in_maps = _host_prep(x, edge_index, W1, b1, W2, b2, Wc1, bc1, Wc2, bc2)
    nc = get_program()
    try:
        res = run_bass_kernel_spmd(nc, in_maps, list(range(NCORES)))
    except Exception:
        # transient device states (e.g. NRT_EXEC_UNIT_UNRECOVERABLE after a
        # wedged prior run) usually clear on retry
        import time as _t

        _t.sleep(2.0)
        res = run_bass_kernel_spmd(nc, in_maps, list(range(NCORES)))
    if _res_out is not None:
        _res_out.append(res)

    node_out = np.concatenate(
        [res.results[c]["node_rows"] for c in range(NCORES)], axis=0
    )
    edge_out = (
        np.concatenate(
            [res.results[c]["edge_rows"] for c in range(NCORES)], axis=0
        )
        .astype(np.float32)
        .reshape(-1)
    )
    ar = np.arange(N, dtype=np.int32)
    full_edge_index = np.stack([np.repeat(ar, N), np.tile(ar, N)])
    return node_out, edge_out, full_edge_index


# revision 39
# speedup vs baseline: 1.0590x; 1.0173x over previous
"""Trainium2 Bass kernel for EnhancedGNN (2-layer GCN + all-pairs edge MLP).

Math (N=2048 nodes, F=128 in-features, H=16 hidden):
  h        = relu(Ahat @ (x @ W1) + b1)            [N, 16]
  node_out = Ahat @ (h @ W2) + b2                  [N, 2]
  E[i,j]   = sigmoid(relu([h_i, h_j] @ Wc1 + bc1) @ Wc2 + bc2)   [N, N]
  full_edge_index = all-pairs (row-major)          [2, N^2]
with Ahat = D^-1/2 (A + I) D^-1/2 built from edge_index (in-degree by dst).

Key algebra: [h_i, h_j] @ Wc1 = (h @ Wc1_top)[i] + (h @ Wc1_bot)[j], so the
N^2 x 32 pair-feature matmul collapses to an outer sum of two [N, 16] tables:
  E[i,j] = sigmoid( sum_k Wc2[k] * relu(A'[i,k] + B[j,k]) + bc2 )
  A' = h @ Wc1_top + bc1,  B = h @ Wc1_bot.

Device strategy per core (rows of E sharded, 256 rows/core):
  * conv1 replicated (every core needs the full node table for the B side),
    streamed in 4 column-chunks; all matmul operands fp16, fp32 PSUM.
  * Edge MLP on a (k4, i32) partition layout: p = k4*32 + i32.  For each
    32-row group and k-quadruple kg: DVE tensor_scalar computes
    R[p,j] = fp16(max(B[j, kg*4+k4] + A'[row(i32), kg*4+k4], 0)) in 4x mode,
    then one PE matmul with the block-diagonal stationary
    Wc2sel4[kg][p, i32'] = Wc2[kg*4+k4] * (i32==i32') accumulates the
    k-contraction straight into PSUM (M=32, legal base partitions 0/32/64;
    rows 96:128 use a second PSUM tile).  Sigmoid runs on ACT out of PSUM.
  * Per-core variation lives entirely in input data (AhatT_own slice);
    the program is identical on all 8 cores.
"""
import sys

import numpy as np

if "/opt/trn_rl_repo" not in sys.path:
    sys.path.insert(0, "/opt/trn_rl_repo")

import concourse.bass as bass
import concourse.tile as tile
from concourse import bacc, mybir
from concourse.bass_utils import run_bass_kernel_spmd

N = 2048
F_IN = 128
H = 16
NCORES = 8
ROWS = N // NCORES          # 256 rows of E per core
NCHUNK = 4
CHUNK = N // NCHUNK         # 512
NCB = N // 128              # 16 contraction blocks
f16 = mybir.dt.float16
f32 = mybir.dt.float32
f8 = mybir.dt.float8e4
CF16 = 2 * H + 512 + 130   # packed f16 consts: W1|Wtopb|Wbot4|Wc2sel4|W2
CF32 = 280                   # packed f32 consts: b1col|b2bc|bc2col

_PROG_CACHE = {}


def _build_program():
    nc = bacc.Bacc("TRN2")

    # AhatT pre-tiled host-side: [chunk, cb, 128, CHUNK] so each (chunk, cb)
    # tile is one contiguous 128KB DRAM read.
    AhatT_d = nc.declare_dram_parameter(
        "AhatT_t", [NCHUNK, 128, NCB, CHUNK], f8, isOutput=False
    )
    Ahown_d = nc.declare_dram_parameter(
        "AhatT_own_t", [128, NCB, ROWS], f8, isOutput=False
    )
    xT_d = nc.declare_dram_parameter("xT", [F_IN, N], f16, isOutput=False)
    cf16_d = nc.declare_dram_parameter("cf16", [128, CF16], f16, isOutput=False)
    cf32_d = nc.declare_dram_parameter("cf32", [128, CF32], f32, isOutput=False)
    dinvrow_d = nc.declare_dram_parameter("dinvrow", [1, N], f32, isOutput=False)
    dinvown_d = nc.declare_dram_parameter("dinvown", [1, ROWS], f32, isOutput=False)
    edge_d = nc.declare_dram_parameter("edge_rows", [ROWS, N], f16, isOutput=True)
    node_d = nc.declare_dram_parameter("node_rows", [2, ROWS], f32, isOutput=True)

    with tile.TileContext(nc) as tc:
        with (
            tc.tile_pool(name="singles", bufs=1) as singles,
            tc.tile_pool(name="ah", bufs=3) as ahpool,
            tc.tile_pool(name="bbf", bufs=2) as bbfpool,
            tc.tile_pool(name="rt", bufs=4) as rtpool,
            tc.tile_pool(name="eo", bufs=3) as eopool,
            tc.tile_pool(name="ps_small", bufs=2, space="PSUM") as ps_small,
            tc.tile_pool(name="ps_bb", bufs=2, space="PSUM") as ps_bb,
            tc.tile_pool(name="ps_la", bufs=2, space="PSUM") as ps_la,
            tc.tile_pool(name="ps_lb", bufs=2, space="PSUM") as ps_lb,
        ):
            # ---- input DMAs --------------------------------------------
            xT_sb = singles.tile([F_IN, N], f16)
            nc.sync.dma_start(out=xT_sb, in_=xT_d[:])
            cf16_sb = singles.tile([128, CF16], f16)
            nc.sync.dma_start(out=cf16_sb, in_=cf16_d[:])
            cf32_sb = singles.tile([128, CF32], f32)
            nc.sync.dma_start(out=cf32_sb, in_=cf32_d[:])
            dinvrow_sb = singles.tile([H, N], f32)
            _dr = dinvrow_d[:]
            nc.gpsimd.dma_start(
                out=dinvrow_sb,
                in_=bass.AP(
                    tensor=_dr.tensor,
                    offset=_dr.offset,
                    ap=[[0, H], [1, N]],
                ),
            )
            dinvown_sb = singles.tile([H, ROWS], f32)
            _do = dinvown_d[:]
            nc.gpsimd.dma_start(
                out=dinvown_sb,
                in_=bass.AP(
                    tensor=_do.tensor,
                    offset=_do.offset,
                    ap=[[0, H], [1, ROWS]],
                ),
            )
            W1_sb = cf16_sb[:, 0:H]
            Wtopb_sb = cf16_sb[0 : H + 1, H : 2 * H]
            Wbot4_sb = cf16_sb[0:H, 2 * H : 2 * H + 512].rearrange(
                "p (g n) -> p g n", g=4
            )
            Wc2sel4_sb = cf16_sb[:, 2 * H + 512 : 2 * H + 512 + 128].rearrange(
                "p (g n) -> p g n", g=4
            )
            W2_sb = cf16_sb[0:H, 2 * H + 512 + 128 : 2 * H + 512 + 130]
            b1col_sb = cf32_sb[0:H, 0:1]
            b2bc_sb = cf32_sb[:, 1:3]
            bc2col_sb = cf32_sb[:, 3:4]
            dinvcols_sb = cf32_sb[:, 4:20]
            dinvocol_sb = cf32_sb[:, 20:22]
            dinvexp_sb = cf32_sb[:, 22:278]  # [p, cb*16+k] = dinv[cb*128+p]
            b2row_sb = cf32_sb[0:2, 278:279]     # b2 as per-partition [2,1]

            # Ahat^T own columns, packed [c_local(128), (cb, i_own)]
            ahown_sb = singles.tile([128, NCB, ROWS], f8)
            nc.scalar.dma_start(out=ahown_sb, in_=Ahown_d[:])

            # ---- xw = x @ W1, packed [c_local, (cb, k)] -----------------
            ps_xw = ps_small.tile([128, NCB * H], f32, tag="small")
            for cb in range(NCB):
                nc.tensor.matmul(
                    ps_xw[:, cb * H : (cb + 1) * H],
                    xT_sb[:, cb * 128 : (cb + 1) * 128],
                    W1_sb,
                    start=True,
                    stop=True,
                )
            xw_sb = singles.tile([128, NCB * H], f16)
            nc.vector.tensor_tensor(
                out=xw_sb, in0=ps_xw, in1=dinvexp_sb, op=mybir.AluOpType.mult
            )

            # ---- conv1 on own columns -> hT_own [17, 256] ---------------
            ps_hown = ps_small.tile([H, ROWS], f32, tag="small")
            for cb in range(NCB):
                nc.tensor.matmul(
                    ps_hown,
                    xw_sb[:, cb * H : (cb + 1) * H],
                    ahown_sb[:, cb, :],
                    start=(cb == 0),
                    stop=(cb == NCB - 1),
                )
            hTown_sb = singles.tile([H + 1, ROWS], f16)
            nc.vector.memset(hTown_sb, 1.0)  # row 16 stays all-ones (bias row)
            hraw_own = singles.tile([H, ROWS], f32)
            nc.vector.tensor_tensor(
                out=hraw_own,
                in0=ps_hown,
                in1=dinvown_sb,
                op=mybir.AluOpType.mult,
            )
            nc.scalar.activation(
                out=hTown_sb[0:H, :],
                in_=hraw_own,
                func=mybir.ActivationFunctionType.Relu,
                bias=b1col_sb,
                scale=1.0,
            )

            # ---- A'_own[i_local, k] per row-block, then rearrange -------
            # A3[:, rb*16+k] = A'_rb ; A4[p, k4*8+rb*4+kg] = A3[p, rb*16+kg*4+k4]
            # T4[k4*32+i32, g32*8+rb*4+kg] = A4[g32*32+i32, k4*8+rb*4+kg]
            A3 = singles.tile([128, 32], f32)
            for rb in range(2):
                ps_ap = ps_small.tile([128, H], f32, tag="small")
                nc.tensor.matmul(
                    ps_ap,
                    hTown_sb[:, rb * 128 : (rb + 1) * 128],
                    Wtopb_sb,
                    start=True,
                    stop=True,
                )
                nc.vector.tensor_copy(A3[:, rb * H : (rb + 1) * H], ps_ap)
            A4 = singles.tile([128, 32], f32)
            a3perm = bass.AP(
                tensor=A3.tensor,
                offset=A3.offset,
                ap=[[A3.ap[0][0], 128], [1, 4], [16, 2], [4, 4]],
            )
            nc.vector.tensor_copy(A4, a3perm)
            T4_0 = singles.tile([128, 8], f32)
            T4_1 = singles.tile([128, 8], f32)
            T4_2 = singles.tile([128, 8], f32)
            T4_3 = singles.tile([128, 8], f32)
            T4g = [T4_0, T4_1, T4_2, T4_3]
            astr = A4.ap[0][0]
            for g32 in range(4):
                for k4 in range(4):
                    srcap = bass.AP(
                        tensor=A4.tensor,
                        offset=A4.offset + g32 * 32 * astr + k4 * 8,
                        ap=[[astr, 32], [1, 8]],
                    )
                    nc.gpsimd.dma_start(
                        out=T4g[g32][k4 * 32 : (k4 + 1) * 32, :],
                        in_=srcap,
                    )

            # ---- conv1 full (chunked) + B tables + edge MLP -------------
            # Superchunks of 1024 columns: TS runs [128, 1024] (one per
            # (g32, kg)), feeding two 512-wide PE matmuls.
            hT_sb = singles.tile([H, N], f16)
            for sc in range(2):
                bb4_t = bbfpool.tile([128, 4, 2 * CHUNK], f16, tag="bbf")
                for half in range(2):
                    ch = sc * 2 + half
                    cs = ch * CHUNK
                    ah_t = ahpool.tile([128, NCB, CHUNK], f8, tag="ah")
                    nc.sync.dma_start(out=ah_t, in_=AhatT_d[ch])
                    ps_h = ps_small.tile([H, CHUNK], f32, tag="small")
                    for cb in range(NCB):
                        nc.tensor.matmul(
                            ps_h,
                            xw_sb[:, cb * H : (cb + 1) * H],
                            ah_t[:, cb, :],
                            start=(cb == 0),
                            stop=(cb == NCB - 1),
                        )
                    hraw_t = eopool.tile([H, CHUNK], f32, tag="hraw")
                    nc.vector.tensor_tensor(
                        out=hraw_t,
                        in0=ps_h,
                        in1=dinvrow_sb[:, cs : cs + CHUNK],
                        op=mybir.AluOpType.mult,
                    )
                    nc.scalar.activation(
                        out=hT_sb[:, cs : cs + CHUNK],
                        in_=hraw_t,
                        func=mybir.ActivationFunctionType.Relu,
                        bias=b1col_sb,
                        scale=1.0,
                    )
                    for kg in range(4):
                        ps_b = ps_bb.tile([128, CHUNK], f32, tag="bb")
                        nc.tensor.matmul(
                            ps_b,
                            Wbot4_sb[:, kg, :],
                            hT_sb[:, cs : cs + CHUNK],
                            start=True,
                            stop=True,
                        )
                        nc.scalar.copy(
                            bb4_t[:, kg, half * CHUNK : (half + 1) * CHUNK], ps_b
                        )

                for rb in range(2):
                    ps_a0 = ps_la.tile([96, CHUNK], f32, tag="la")
                    ps_a1 = ps_la.tile([96, CHUNK], f32, tag="la")
                    ps_b20 = ps_lb.tile([32, CHUNK], f32, tag="lb")
                    ps_b21 = ps_lb.tile([32, CHUNK], f32, tag="lb")
                    ps_a = [ps_a0, ps_a1]
                    ps_b2 = [ps_b20, ps_b21]
                    for g32 in range(4):
                        for kg in range(4):
                            r_t = rtpool.tile([128, 2 * CHUNK], f16, tag="r")
                            nc.vector.tensor_scalar(
                                out=r_t,
                                in0=bb4_t[:, kg, :],
                                scalar1=T4g[g32][
                                    :,
                                    rb * 4 + kg : rb * 4 + kg + 1,
                                ],
                                scalar2=0.0,
                                op0=mybir.AluOpType.add,
                                op1=mybir.AluOpType.max,
                            )
                            for half in range(2):
                                out_ps = (
                                    ps_a[half][g32 * 32 : (g32 + 1) * 32, :]
                                    if g32 < 3
                                    else ps_b2[half]
                                )
                                nc.tensor.matmul(
                                    out_ps,
                                    Wc2sel4_sb[:, kg, :],
                                    r_t[:, half * CHUNK : (half + 1) * CHUNK],
                                    start=(kg == 0),
                                    stop=(kg == 3),
                                )
                    for half in range(2):
                        cs = (sc * 2 + half) * CHUNK
                        e_t = eopool.tile([96, CHUNK], f16, tag="e")
                        nc.scalar.activation(
                            out=e_t,
                            in_=ps_a[half],
                            func=mybir.ActivationFunctionType.Sigmoid,
                            bias=bc2col_sb[0:96, :],
                            scale=1.0,
                        )
                        e_t2 = eopool.tile([32, CHUNK], f16, tag="e2")
                        nc.scalar.activation(
                            out=e_t2,
                            in_=ps_b2[half],
                            func=mybir.ActivationFunctionType.Sigmoid,
                            bias=bc2col_sb[0:32, :],
                            scale=1.0,
                        )
                        nc.scalar.dma_start(
                            out=edge_d[rb * 128 : rb * 128 + 96, cs : cs + CHUNK],
                            in_=e_t,
                        )
                        nc.scalar.dma_start(
                            out=edge_d[rb * 128 + 96 : (rb + 1) * 128, cs : cs + CHUNK],
                            in_=e_t2,
                        )

            # ---- conv2: hw2 = h @ W2 packed [c_local, (cb, o)] ----------
            h2T_sb = singles.tile([H, N], f16)
            nc.vector.tensor_tensor(
                out=h2T_sb,
                in0=hT_sb,
                in1=dinvrow_sb,
                op=mybir.AluOpType.mult,
            )
            ps_hw2 = ps_small.tile([128, NCB * 2], f32, tag="small")
            for cb in range(NCB):
                nc.tensor.matmul(
                    ps_hw2[:, cb * 2 : (cb + 1) * 2],
                    h2T_sb[:, cb * 128 : (cb + 1) * 128],
                    W2_sb,
                    start=True,
                    stop=True,
                )
            hw2_sb = singles.tile([128, NCB * 2], f16)
            nc.vector.tensor_copy(hw2_sb, ps_hw2)

            ps_no = ps_small.tile([2, ROWS], f32, tag="small")
            for cb in range(NCB):
                nc.tensor.matmul(
                    ps_no,
                    hw2_sb[:, cb * 2 : (cb + 1) * 2],
                    ahown_sb[:, cb, :],
                    start=(cb == 0),
                    stop=(cb == NCB - 1),
                )
            no_sb = eopool.tile([2, ROWS], f32, tag="no")
            nc.vector.tensor_tensor(
                out=no_sb, in0=ps_no, in1=dinvown_sb[0:2, :], op=mybir.AluOpType.mult
            )
            nc.vector.tensor_scalar(
                out=no_sb,
                in0=no_sb,
                scalar1=b2row_sb,
                scalar2=None,
                op0=mybir.AluOpType.add,
            )
            nc.scalar.dma_start(out=node_d[:], in_=no_sb)

    nc.finalize()
    return nc


def get_program():
    if "nc" not in _PROG_CACHE:
        _PROG_CACHE["nc"] = _build_program()
    return _PROG_CACHE["nc"]


def _host_prep(x, edge_index, W1, b1, W2, b2, Wc1, bc1, Wc2, bc2):
    x = np.asarray(x, dtype=np.float32)
    ei = np.asarray(edge_index)
    src = ei[0].astype(np.int64)
    dst = ei[1].astype(np.int64)
    W1 = np.asarray(W1, np.float32)
    b1 = np.asarray(b1, np.float32)
    W2 = np.asarray(W2, np.float32)
    b2 = np.asarray(b2, np.float32)
    Wc1 = np.asarray(Wc1, np.float32)
    bc1 = np.asarray(bc1, np.float32)
    Wc2 = np.asarray(Wc2, np.float32)
    bc2 = np.asarray(bc2, np.float32)

    deg = (np.bincount(dst, minlength=N) + 1).astype(np.float32)
    dinv = 1.0 / np.sqrt(deg)
    CT = np.zeros((N, N), np.float32)
    np.add.at(CT, (src, dst), 1.0)
    idx = np.arange(N)
    CT[idx, idx] += 1.0
    assert CT.max() <= 16, "edge multiplicity too high for exact fp8 counts"
    f8np = mybir.dt.np(f8)
    CT8 = CT.astype(f8np)

    xT16 = np.ascontiguousarray(x.T).astype(np.float16)
    W1_16 = W1.astype(np.float16)
    Wtopb = np.concatenate([Wc1[:H], bc1[None, :]], axis=0).astype(np.float16)
    # Wbotrep4[f, kg, k4*32+i32] = Wc1_bot[f, kg*4+k4]
    Wbot = Wc1[H:].astype(np.float16)            # [16, 16]
    Wbotrep4 = np.zeros((H, 4, 128), np.float16)
    for kg in range(4):
        for k4 in range(4):
            Wbotrep4[:, kg, k4 * 32 : (k4 + 1) * 32] = Wbot[:, kg * 4 + k4][:, None]
    # Wc2sel4[k4*32+i32, kg, i32'] = Wc2[kg*4+k4] * (i32 == i32')
    Wc2sel4 = np.zeros((128, 4, 32), np.float16)
    eye32 = np.eye(32, dtype=np.float16)
    for kg in range(4):
        for k4 in range(4):
            Wc2sel4[k4 * 32 : (k4 + 1) * 32, kg, :] = (
                np.float16(Wc2[kg * 4 + k4, 0]) * eye32
            )
    W2_16 = W2.astype(np.float16)
    b1col = b1.reshape(H, 1).astype(np.float32)
    b2bc = np.tile(b2.reshape(1, 2), (128, 1)).astype(np.float32)
    bc2col = np.full((128, 1), bc2[0], np.float32)

    # pre-tile: AhatT_t[ch, p, cb, j] = CT8[cb*128+p, ch*CHUNK+j]
    AhatT_t = np.ascontiguousarray(
        CT8.reshape(NCB, 128, NCHUNK, CHUNK).transpose(2, 1, 0, 3)
    )
    cf16 = np.zeros((128, CF16), np.float16)
    cf16[:, 0:H] = W1_16
    cf16[0 : H + 1, H : 2 * H] = Wtopb
    cf16[0:H, 2 * H : 2 * H + 512] = Wbotrep4.reshape(H, 512)
    cf16[:, 2 * H + 512 : 2 * H + 512 + 128] = Wc2sel4.reshape(128, 128)
    cf16[0:H, 2 * H + 512 + 128 : 2 * H + 512 + 130] = W2_16
    cf32 = np.zeros((128, CF32), np.float32)
    cf32[0:H, 0:1] = b1col
    cf32[:, 1:3] = b2bc
    cf32[:, 3:4] = bc2col
    cf32[:, 4:20] = dinv.reshape(NCB, 128).T  # dinvcols[p, cb] = dinv[cb*128+p]
    cf32[:, 22:278] = np.repeat(dinv.reshape(NCB, 128).T, H, axis=1)
    cf32[0:2, 278] = b2
    shared = {
        "AhatT_t": AhatT_t,
        "xT": xT16,
        "cf16": cf16,
        "dinvrow": dinv.reshape(1, N),
    }
    in_maps = []
    for c in range(NCORES):
        m = dict(shared)
        m["AhatT_own_t"] = np.ascontiguousarray(
            CT8[:, c * ROWS : (c + 1) * ROWS]
            .reshape(NCB, 128, ROWS)
            .transpose(1, 0, 2)
        )
        dn = dinv[c * ROWS : (c + 1) * ROWS]
        m["dinvown"] = np.ascontiguousarray(dn.reshape(1, ROWS))
        cfc = cf32.copy()
        cfc[:, 20:22] = dn.reshape(2, 128).T  # dinvocol[p, rb] = dinv[own + rb*128+p]
        m["cf32"] = cfc
        in_maps.append(m)
    return in_maps


def kernel(x, edge_index, W1, b1, W2, b2, Wc1, bc1, Wc2, bc2, _res_out=None):
    in_maps = _host_prep(x, edge_index, W1, b1, W2, b2, Wc1, bc1, Wc2, bc2)
    nc = get_program()
    try:
        res = run_bass_kernel_spmd(nc, in_maps, list(range(NCORES)))
    except Exception:
        # transient device states (e.g. NRT_EXEC_UNIT_UNRECOVERABLE after a
        # wedged prior run) usually clear on retry
        import time as _t

        _t.sleep(2.0)
        res = run_bass_kernel_spmd(nc, in_maps, list(range(NCORES)))
    if _res_out is not None:
        _res_out.append(res)

    node_out = np.concatenate(
        [res.results[c]["node_rows"].T for c in range(NCORES)], axis=0
    )
    edge_out = (
        np.concatenate(
            [res.results[c]["edge_rows"] for c in range(NCORES)], axis=0
        )
        .astype(np.float32)
        .reshape(-1)
    )
    ar = np.arange(N, dtype=np.int32)
    full_edge_index = np.stack([np.repeat(ar, N), np.tile(ar, N)])
    return node_out, edge_out, full_edge_index


# revision 40
# speedup vs baseline: 1.0615x; 1.0024x over previous
"""Trainium2 Bass kernel for EnhancedGNN (2-layer GCN + all-pairs edge MLP).

Math (N=2048 nodes, F=128 in-features, H=16 hidden):
  h        = relu(Ahat @ (x @ W1) + b1)            [N, 16]
  node_out = Ahat @ (h @ W2) + b2                  [N, 2]
  E[i,j]   = sigmoid(relu([h_i, h_j] @ Wc1 + bc1) @ Wc2 + bc2)   [N, N]
  full_edge_index = all-pairs (row-major)          [2, N^2]
with Ahat = D^-1/2 (A + I) D^-1/2 built from edge_index (in-degree by dst).

Key algebra: [h_i, h_j] @ Wc1 = (h @ Wc1_top)[i] + (h @ Wc1_bot)[j], so the
N^2 x 32 pair-feature matmul collapses to an outer sum of two [N, 16] tables:
  E[i,j] = sigmoid( sum_k Wc2[k] * relu(A'[i,k] + B[j,k]) + bc2 )
  A' = h @ Wc1_top + bc1,  B = h @ Wc1_bot.

Device strategy per core (rows of E sharded, 256 rows/core):
  * conv1 replicated (every core needs the full node table for the B side),
    streamed in 4 column-chunks; all matmul operands fp16, fp32 PSUM.
  * Edge MLP on a (k4, i32) partition layout: p = k4*32 + i32.  For each
    32-row group and k-quadruple kg: DVE tensor_scalar computes
    R[p,j] = fp16(max(B[j, kg*4+k4] + A'[row(i32), kg*4+k4], 0)) in 4x mode,
    then one PE matmul with the block-diagonal stationary
    Wc2sel4[kg][p, i32'] = Wc2[kg*4+k4] * (i32==i32') accumulates the
    k-contraction straight into PSUM (M=32, legal base partitions 0/32/64;
    rows 96:128 use a second PSUM tile).  Sigmoid runs on ACT out of PSUM.
  * Per-core variation lives entirely in input data (AhatT_own slice);
    the program is identical on all 8 cores.
"""
import sys

import numpy as np

if "/opt/trn_rl_repo" not in sys.path:
    sys.path.insert(0, "/opt/trn_rl_repo")

import concourse.bass as bass
import concourse.tile as tile
from concourse import bacc, mybir
from concourse.bass_utils import run_bass_kernel_spmd

N = 2048
F_IN = 128
H = 16
NCORES = 8
ROWS = N // NCORES          # 256 rows of E per core
NCHUNK = 4
CHUNK = N // NCHUNK         # 512
NCB = N // 128              # 16 contraction blocks
f16 = mybir.dt.float16
f32 = mybir.dt.float32
f8 = mybir.dt.float8e4
CF16 = 2 * H + 512 + 130   # packed f16 consts: W1|Wtopb|Wbot4|Wc2sel4|W2
CF32 = 280                   # packed f32 consts: b1col|b2bc|bc2col

_PROG_CACHE = {}


def _build_program():
    nc = bacc.Bacc("TRN2")

    # AhatT pre-tiled host-side: [chunk, cb, 128, CHUNK] so each (chunk, cb)
    # tile is one contiguous 128KB DRAM read.
    AhatT_d = nc.declare_dram_parameter(
        "AhatT_t", [NCHUNK, 128, NCB, CHUNK], f8, isOutput=False
    )
    Ahown_d = nc.declare_dram_parameter(
        "AhatT_own_t", [128, NCB, ROWS], f8, isOutput=False
    )
    xT_d = nc.declare_dram_parameter("xT", [F_IN, N], f16, isOutput=False)
    cf16_d = nc.declare_dram_parameter("cf16", [128, CF16], f16, isOutput=False)
    cf32_d = nc.declare_dram_parameter("cf32", [128, CF32], f32, isOutput=False)
    dinvrow_d = nc.declare_dram_parameter("dinvrow", [1, N], f32, isOutput=False)
    dinvown_d = nc.declare_dram_parameter("dinvown", [1, ROWS], f32, isOutput=False)
    edge_d = nc.declare_dram_parameter("edge_rows", [ROWS, N], f16, isOutput=True)
    node_d = nc.declare_dram_parameter("node_rows", [2, ROWS], f32, isOutput=True)

    with tile.TileContext(nc) as tc:
        with (
            tc.tile_pool(name="singles", bufs=1) as singles,
            tc.tile_pool(name="ah", bufs=3) as ahpool,
            tc.tile_pool(name="bbf", bufs=2) as bbfpool,
            tc.tile_pool(name="rt", bufs=4) as rtpool,
            tc.tile_pool(name="eo", bufs=3) as eopool,
            tc.tile_pool(name="ps_small", bufs=2, space="PSUM") as ps_small,
            tc.tile_pool(name="ps_bb", bufs=2, space="PSUM") as ps_bb,
            tc.tile_pool(name="ps_la", bufs=2, space="PSUM") as ps_la,
            tc.tile_pool(name="ps_lb", bufs=2, space="PSUM") as ps_lb,
        ):
            # ---- input DMAs --------------------------------------------
            xT_sb = singles.tile([F_IN, N], f16)
            nc.sync.dma_start(out=xT_sb, in_=xT_d[:])
            cf16_sb = singles.tile([128, CF16], f16)
            nc.sync.dma_start(out=cf16_sb, in_=cf16_d[:])
            cf32_sb = singles.tile([128, CF32], f32)
            nc.sync.dma_start(out=cf32_sb, in_=cf32_d[:])
            dinvrow_sb = singles.tile([H, N], f32)
            _dr = dinvrow_d[:]
            nc.gpsimd.dma_start(
                out=dinvrow_sb,
                in_=bass.AP(
                    tensor=_dr.tensor,
                    offset=_dr.offset,
                    ap=[[0, H], [1, N]],
                ),
            )
            dinvown_sb = singles.tile([H, ROWS], f32)
            _do = dinvown_d[:]
            nc.gpsimd.dma_start(
                out=dinvown_sb,
                in_=bass.AP(
                    tensor=_do.tensor,
                    offset=_do.offset,
                    ap=[[0, H], [1, ROWS]],
                ),
            )
            W1_sb = cf16_sb[:, 0:H]
            Wtopb_sb = cf16_sb[0 : H + 1, H : 2 * H]
            Wbot4_sb = cf16_sb[0:H, 2 * H : 2 * H + 512].rearrange(
                "p (g n) -> p g n", g=4
            )
            Wc2sel4_sb = cf16_sb[:, 2 * H + 512 : 2 * H + 512 + 128].rearrange(
                "p (g n) -> p g n", g=4
            )
            W2_sb = cf16_sb[0:H, 2 * H + 512 + 128 : 2 * H + 512 + 130]
            b1col_sb = cf32_sb[0:H, 0:1]
            b2bc_sb = cf32_sb[:, 1:3]
            bc2col_sb = cf32_sb[:, 3:4]
            dinvcols_sb = cf32_sb[:, 4:20]
            dinvocol_sb = cf32_sb[:, 20:22]
            dinvexp_sb = cf32_sb[:, 22:278]  # [p, cb*16+k] = dinv[cb*128+p]
            b2row_sb = cf32_sb[0:2, 278:279]     # b2 as per-partition [2,1]

            # Ahat^T own columns, packed [c_local(128), (cb, i_own)]
            ahown_sb = singles.tile([128, NCB, ROWS], f8)
            nc.scalar.dma_start(out=ahown_sb, in_=Ahown_d[:])

            # PE clock warmup: dummy matmuls on the const tile while xT
            # streams in, so real matmuls start at ramped pstate.
            warm_ps = ps_la.tile([32, CHUNK], f32, tag="la")
            for _ in range(8):
                nc.tensor.matmul(
                    warm_ps,
                    cf16_sb[:, 0:32],
                    cf16_sb[:, 0:CHUNK],
                    start=True,
                    stop=True,
                )
            warm_sb = eopool.tile([32, 16], f32, tag="warm")
            nc.vector.tensor_copy(warm_sb, warm_ps[:, 0:16])

            # ---- xw = x @ W1, packed [c_local, (cb, k)] -----------------
            ps_xw = ps_small.tile([128, NCB * H], f32, tag="small")
            for cb in range(NCB):
                nc.tensor.matmul(
                    ps_xw[:, cb * H : (cb + 1) * H],
                    xT_sb[:, cb * 128 : (cb + 1) * 128],
                    W1_sb,
                    start=True,
                    stop=True,
                )
            xw_sb = singles.tile([128, NCB * H], f16)
            nc.vector.tensor_tensor(
                out=xw_sb, in0=ps_xw, in1=dinvexp_sb, op=mybir.AluOpType.mult
            )

            # ---- conv1 on own columns -> hT_own [17, 256] ---------------
            ps_hown = ps_small.tile([H, ROWS], f32, tag="small")
            for cb in range(NCB):
                nc.tensor.matmul(
                    ps_hown,
                    xw_sb[:, cb * H : (cb + 1) * H],
                    ahown_sb[:, cb, :],
                    start=(cb == 0),
                    stop=(cb == NCB - 1),
                )
            hTown_sb = singles.tile([H + 1, ROWS], f16)
            nc.vector.memset(hTown_sb, 1.0)  # row 16 stays all-ones (bias row)
            hraw_own = singles.tile([H, ROWS], f32)
            nc.vector.tensor_tensor(
                out=hraw_own,
                in0=ps_hown,
                in1=dinvown_sb,
                op=mybir.AluOpType.mult,
            )
            nc.scalar.activation(
                out=hTown_sb[0:H, :],
                in_=hraw_own,
                func=mybir.ActivationFunctionType.Relu,
                bias=b1col_sb,
                scale=1.0,
            )

            # ---- A'_own[i_local, k] per row-block, then rearrange -------
            # A3[:, rb*16+k] = A'_rb ; A4[p, k4*8+rb*4+kg] = A3[p, rb*16+kg*4+k4]
            # T4[k4*32+i32, g32*8+rb*4+kg] = A4[g32*32+i32, k4*8+rb*4+kg]
            A3 = singles.tile([128, 32], f32)
            for rb in range(2):
                ps_ap = ps_small.tile([128, H], f32, tag="small")
                nc.tensor.matmul(
                    ps_ap,
                    hTown_sb[:, rb * 128 : (rb + 1) * 128],
                    Wtopb_sb,
                    start=True,
                    stop=True,
                )
                nc.vector.tensor_copy(A3[:, rb * H : (rb + 1) * H], ps_ap)
            A4 = singles.tile([128, 32], f32)
            a3perm = bass.AP(
                tensor=A3.tensor,
                offset=A3.offset,
                ap=[[A3.ap[0][0], 128], [1, 4], [16, 2], [4, 4]],
            )
            nc.vector.tensor_copy(A4, a3perm)
            T4_0 = singles.tile([128, 8], f32)
            T4_1 = singles.tile([128, 8], f32)
            T4_2 = singles.tile([128, 8], f32)
            T4_3 = singles.tile([128, 8], f32)
            T4g = [T4_0, T4_1, T4_2, T4_3]
            astr = A4.ap[0][0]
            for g32 in range(4):
                for k4 in range(4):
                    srcap = bass.AP(
                        tensor=A4.tensor,
                        offset=A4.offset + g32 * 32 * astr + k4 * 8,
                        ap=[[astr, 32], [1, 8]],
                    )
                    nc.gpsimd.dma_start(
                        out=T4g[g32][k4 * 32 : (k4 + 1) * 32, :],
                        in_=srcap,
                    )

            # ---- conv1 full (chunked) + B tables + edge MLP -------------
            # Superchunks of 1024 columns: TS runs [128, 1024] (one per
            # (g32, kg)), feeding two 512-wide PE matmuls.
            hT_sb = singles.tile([H, N], f16)
            for sc in range(2):
                bb4_t = bbfpool.tile([128, 4, 2 * CHUNK], f16, tag="bbf")
                for half in range(2):
                    ch = sc * 2 + half
                    cs = ch * CHUNK
                    ah_t = ahpool.tile([128, NCB, CHUNK], f8, tag="ah")
                    nc.sync.dma_start(out=ah_t, in_=AhatT_d[ch])
                    ps_h = ps_small.tile([H, CHUNK], f32, tag="small")
                    for cb in range(NCB):
                        nc.tensor.matmul(
                            ps_h,
                            xw_sb[:, cb * H : (cb + 1) * H],
                            ah_t[:, cb, :],
                            start=(cb == 0),
                            stop=(cb == NCB - 1),
                        )
                    hraw_t = eopool.tile([H, CHUNK], f32, tag="hraw")
                    nc.vector.tensor_tensor(
                        out=hraw_t,
                        in0=ps_h,
                        in1=dinvrow_sb[:, cs : cs + CHUNK],
                        op=mybir.AluOpType.mult,
                    )
                    nc.scalar.activation(
                        out=hT_sb[:, cs : cs + CHUNK],
                        in_=hraw_t,
                        func=mybir.ActivationFunctionType.Relu,
                        bias=b1col_sb,
                        scale=1.0,
                    )
                    for kg in range(4):
                        ps_b = ps_bb.tile([128, CHUNK], f32, tag="bb")
                        nc.tensor.matmul(
                            ps_b,
                            Wbot4_sb[:, kg, :],
                            hT_sb[:, cs : cs + CHUNK],
                            start=True,
                            stop=True,
                        )
                        nc.scalar.copy(
                            bb4_t[:, kg, half * CHUNK : (half + 1) * CHUNK], ps_b
                        )

                for rb in range(2):
                    ps_a0 = ps_la.tile([96, CHUNK], f32, tag="la")
                    ps_a1 = ps_la.tile([96, CHUNK], f32, tag="la")
                    ps_b20 = ps_lb.tile([32, CHUNK], f32, tag="lb")
                    ps_b21 = ps_lb.tile([32, CHUNK], f32, tag="lb")
                    ps_a = [ps_a0, ps_a1]
                    ps_b2 = [ps_b20, ps_b21]
                    for g32 in range(4):
                        for kg in range(4):
                            r_t = rtpool.tile([128, 2 * CHUNK], f16, tag="r")
                            nc.vector.tensor_scalar(
                                out=r_t,
                                in0=bb4_t[:, kg, :],
                                scalar1=T4g[g32][
                                    :,
                                    rb * 4 + kg : rb * 4 + kg + 1,
                                ],
                                scalar2=0.0,
                                op0=mybir.AluOpType.add,
                                op1=mybir.AluOpType.max,
                            )
                            for half in range(2):
                                out_ps = (
                                    ps_a[half][g32 * 32 : (g32 + 1) * 32, :]
                                    if g32 < 3
                                    else ps_b2[half]
                                )
                                nc.tensor.matmul(
                                    out_ps,
                                    Wc2sel4_sb[:, kg, :],
                                    r_t[:, half * CHUNK : (half + 1) * CHUNK],
                                    start=(kg == 0),
                                    stop=(kg == 3),
                                )
                    for half in range(2):
                        cs = (sc * 2 + half) * CHUNK
                        e_t = eopool.tile([96, CHUNK], f16, tag="e")
                        nc.scalar.activation(
                            out=e_t,
                            in_=ps_a[half],
                            func=mybir.ActivationFunctionType.Sigmoid,
                            bias=bc2col_sb[0:96, :],
                            scale=1.0,
                        )
                        e_t2 = eopool.tile([32, CHUNK], f16, tag="e2")
                        nc.scalar.activation(
                            out=e_t2,
                            in_=ps_b2[half],
                            func=mybir.ActivationFunctionType.Sigmoid,
                            bias=bc2col_sb[0:32, :],
                            scale=1.0,
                        )
                        nc.scalar.dma_start(
                            out=edge_d[rb * 128 : rb * 128 + 96, cs : cs + CHUNK],
                            in_=e_t,
                        )
                        nc.scalar.dma_start(
                            out=edge_d[rb * 128 + 96 : (rb + 1) * 128, cs : cs + CHUNK],
                            in_=e_t2,
                        )

            # ---- conv2: hw2 = h @ W2 packed [c_local, (cb, o)] ----------
            h2T_sb = singles.tile([H, N], f16)
            nc.vector.tensor_tensor(
                out=h2T_sb,
                in0=hT_sb,
                in1=dinvrow_sb,
                op=mybir.AluOpType.mult,
            )
            ps_hw2 = ps_small.tile([128, NCB * 2], f32, tag="small")
            for cb in range(NCB):
                nc.tensor.matmul(
                    ps_hw2[:, cb * 2 : (cb + 1) * 2],
                    h2T_sb[:, cb * 128 : (cb + 1) * 128],
                    W2_sb,
                    start=True,
                    stop=True,
                )
            hw2_sb = singles.tile([128, NCB * 2], f16)
            nc.vector.tensor_copy(hw2_sb, ps_hw2)

            ps_no = ps_small.tile([2, ROWS], f32, tag="small")
            for cb in range(NCB):
                nc.tensor.matmul(
                    ps_no,
                    hw2_sb[:, cb * 2 : (cb + 1) * 2],
                    ahown_sb[:, cb, :],
                    start=(cb == 0),
                    stop=(cb == NCB - 1),
                )
            no_sb = eopool.tile([2, ROWS], f32, tag="no")
            nc.vector.tensor_tensor(
                out=no_sb, in0=ps_no, in1=dinvown_sb[0:2, :], op=mybir.AluOpType.mult
            )
            nc.vector.tensor_scalar(
                out=no_sb,
                in0=no_sb,
                scalar1=b2row_sb,
                scalar2=None,
                op0=mybir.AluOpType.add,
            )
            nc.scalar.dma_start(out=node_d[:], in_=no_sb)

    nc.finalize()
    return nc


def get_program():
    if "nc" not in _PROG_CACHE:
        _PROG_CACHE["nc"] = _build_program()
    return _PROG_CACHE["nc"]


def _host_prep(x, edge_index, W1, b1, W2, b2, Wc1, bc1, Wc2, bc2):
    x = np.asarray(x, dtype=np.float32)
    ei = np.asarray(edge_index)
    src = ei[0].astype(np.int64)
    dst = ei[1].astype(np.int64)
    W1 = np.asarray(W1, np.float32)
    b1 = np.asarray(b1, np.float32)
    W2 = np.asarray(W2, np.float32)
    b2 = np.asarray(b2, np.float32)
    Wc1 = np.asarray(Wc1, np.float32)
    bc1 = np.asarray(bc1, np.float32)
    Wc2 = np.asarray(Wc2, np.float32)
    bc2 = np.asarray(bc2, np.float32)

    deg = (np.bincount(dst, minlength=N) + 1).astype(np.float32)
    dinv = 1.0 / np.sqrt(deg)
    CT = np.zeros((N, N), np.float32)
    np.add.at(CT, (src, dst), 1.0)
    idx = np.arange(N)
    CT[idx, idx] += 1.0
    assert CT.max() <= 16, "edge multiplicity too high for exact fp8 counts"
    f8np = mybir.dt.np(f8)
    CT8 = CT.astype(f8np)

    xT16 = np.ascontiguousarray(x.T).astype(np.float16)
    W1_16 = W1.astype(np.float16)
    Wtopb = np.concatenate([Wc1[:H], bc1[None, :]], axis=0).astype(np.float16)
    # Wbotrep4[f, kg, k4*32+i32] = Wc1_bot[f, kg*4+k4]
    Wbot = Wc1[H:].astype(np.float16)            # [16, 16]
    Wbotrep4 = np.zeros((H, 4, 128), np.float16)
    for kg in range(4):
        for k4 in range(4):
            Wbotrep4[:, kg, k4 * 32 : (k4 + 1) * 32] = Wbot[:, kg * 4 + k4][:, None]
    # Wc2sel4[k4*32+i32, kg, i32'] = Wc2[kg*4+k4] * (i32 == i32')
    Wc2sel4 = np.zeros((128, 4, 32), np.float16)
    eye32 = np.eye(32, dtype=np.float16)
    for kg in range(4):
        for k4 in range(4):
            Wc2sel4[k4 * 32 : (k4 + 1) * 32, kg, :] = (
                np.float16(Wc2[kg * 4 + k4, 0]) * eye32
            )
    W2_16 = W2.astype(np.float16)
    b1col = b1.reshape(H, 1).astype(np.float32)
    b2bc = np.tile(b2.reshape(1, 2), (128, 1)).astype(np.float32)
    bc2col = np.full((128, 1), bc2[0], np.float32)

    # pre-tile: AhatT_t[ch, p, cb, j] = CT8[cb*128+p, ch*CHUNK+j]
    AhatT_t = np.ascontiguousarray(
        CT8.reshape(NCB, 128, NCHUNK, CHUNK).transpose(2, 1, 0, 3)
    )
    cf16 = np.zeros((128, CF16), np.float16)
    cf16[:, 0:H] = W1_16
    cf16[0 : H + 1, H : 2 * H] = Wtopb
    cf16[0:H, 2 * H : 2 * H + 512] = Wbotrep4.reshape(H, 512)
    cf16[:, 2 * H + 512 : 2 * H + 512 + 128] = Wc2sel4.reshape(128, 128)
    cf16[0:H, 2 * H + 512 + 128 : 2 * H + 512 + 130] = W2_16
    cf32 = np.zeros((128, CF32), np.float32)
    cf32[0:H, 0:1] = b1col
    cf32[:, 1:3] = b2bc
    cf32[:, 3:4] = bc2col
    cf32[:, 4:20] = dinv.reshape(NCB, 128).T  # dinvcols[p, cb] = dinv[cb*128+p]
    cf32[:, 22:278] = np.repeat(dinv.reshape(NCB, 128).T, H, axis=1)
    cf32[0:2, 278] = b2
    shared = {
        "AhatT_t": AhatT_t,
        "xT": xT16,
        "cf16": cf16,
        "dinvrow": dinv.reshape(1, N),
    }
    in_maps = []
    for c in range(NCORES):
        m = dict(shared)
        m["AhatT_own_t"] = np.ascontiguousarray(
            CT8[:, c * ROWS : (c + 1) * ROWS]
            .reshape(NCB, 128, ROWS)
            .transpose(1, 0, 2)
        )
        dn = dinv[c * ROWS : (c + 1) * ROWS]
        m["dinvown"] = np.ascontiguousarray(dn.reshape(1, ROWS))
        cfc = cf32.copy()
        cfc[:, 20:22] = dn.reshape(2, 128).T  # dinvocol[p, rb] = dinv[own + rb*128+p]
        m["cf32"] = cfc
        in_maps.append(m)
    return in_maps


def kernel(x, edge_index, W1, b1, W2, b2, Wc1, bc1, Wc2, bc2, _res_out=None):
    in_maps = _host_prep(x, edge_index, W1, b1, W2, b2, Wc1, bc1, Wc2, bc2)
    nc = get_program()
    try:
        res = run_bass_kernel_spmd(nc, in_maps, list(range(NCORES)))
    except Exception:
        # transient device states (e.g. NRT_EXEC_UNIT_UNRECOVERABLE after a
        # wedged prior run) usually clear on retry
        import time as _t

        _t.sleep(2.0)
        res = run_bass_kernel_spmd(nc, in_maps, list(range(NCORES)))
    if _res_out is not None:
        _res_out.append(res)

    node_out = np.concatenate(
        [res.results[c]["node_rows"].T for c in range(NCORES)], axis=0
    )
    edge_out = (
        np.concatenate(
            [res.results[c]["edge_rows"] for c in range(NCORES)], axis=0
        )
        .astype(np.float32)
        .reshape(-1)
    )
    ar = np.arange(N, dtype=np.int32)
    full_edge_index = np.stack([np.repeat(ar, N), np.tile(ar, N)])
    return node_out, edge_out, full_edge_index


# revision 41
# speedup vs baseline: 1.0885x; 1.0254x over previous
"""Trainium2 Bass kernel for EnhancedGNN (2-layer GCN + all-pairs edge MLP).

Math (N=2048 nodes, F=128 in-features, H=16 hidden):
  h        = relu(Ahat @ (x @ W1) + b1)            [N, 16]
  node_out = Ahat @ (h @ W2) + b2                  [N, 2]
  E[i,j]   = sigmoid(relu([h_i, h_j] @ Wc1 + bc1) @ Wc2 + bc2)   [N, N]
  full_edge_index = all-pairs (row-major)          [2, N^2]
with Ahat = D^-1/2 (A + I) D^-1/2 built from edge_index (in-degree by dst).

Key algebra: [h_i, h_j] @ Wc1 = (h @ Wc1_top)[i] + (h @ Wc1_bot)[j], so the
N^2 x 32 pair-feature matmul collapses to an outer sum of two [N, 16] tables:
  E[i,j] = sigmoid( sum_k Wc2[k] * relu(A'[i,k] + B[j,k]) + bc2 )
  A' = h @ Wc1_top + bc1,  B = h @ Wc1_bot.

Device strategy per core (rows of E sharded, 256 rows/core):
  * conv1 replicated (every core needs the full node table for the B side),
    streamed in 4 column-chunks; all matmul operands fp16, fp32 PSUM.
  * Edge MLP on a (k4, i32) partition layout: p = k4*32 + i32.  For each
    32-row group and k-quadruple kg: DVE tensor_scalar computes
    R[p,j] = fp16(max(B[j, kg*4+k4] + A'[row(i32), kg*4+k4], 0)) in 4x mode,
    then one PE matmul with the block-diagonal stationary
    Wc2sel4[kg][p, i32'] = Wc2[kg*4+k4] * (i32==i32') accumulates the
    k-contraction straight into PSUM (M=32, legal base partitions 0/32/64;
    rows 96:128 use a second PSUM tile).  Sigmoid runs on ACT out of PSUM.
  * Per-core variation lives entirely in input data (AhatT_own slice);
    the program is identical on all 8 cores.
"""
import sys

import numpy as np

if "/opt/trn_rl_repo" not in sys.path:
    sys.path.insert(0, "/opt/trn_rl_repo")

import concourse.bass as bass
import concourse.tile as tile
from concourse import bacc, mybir
from concourse.bass_utils import run_bass_kernel_spmd

N = 2048
F_IN = 128
H = 16
NCORES = 8
ROWS = N // NCORES          # 256 rows of E per core
NCHUNK = 4
CHUNK = N // NCHUNK         # 512
NCB = N // 128              # 16 contraction blocks
f16 = mybir.dt.float16
f32 = mybir.dt.float32
f8 = mybir.dt.float8e4
CF16 = 2 * H + 512 + 130   # packed f16 consts: W1|Wtopb|Wbot4|Wc2sel4|W2
CF32 = 280                   # packed f32 consts: b1col|b2bc|bc2col

_PROG_CACHE = {}


def _build_program():
    nc = bacc.Bacc("TRN2")

    # AhatT pre-tiled host-side: [chunk, cb, 128, CHUNK] so each (chunk, cb)
    # tile is one contiguous 128KB DRAM read.
    AhatT_d = nc.declare_dram_parameter(
        "AhatT_t", [NCHUNK, 128, NCB, CHUNK], f8, isOutput=False
    )
    Ahown_d = nc.declare_dram_parameter(
        "AhatT_own_t", [128, NCB, ROWS], f8, isOutput=False
    )
    xT_d = nc.declare_dram_parameter("xT", [F_IN, N], f16, isOutput=False)
    cf16_d = nc.declare_dram_parameter("cf16", [128, CF16], f16, isOutput=False)
    cf32_d = nc.declare_dram_parameter("cf32", [128, CF32], f32, isOutput=False)
    dinvrow_d = nc.declare_dram_parameter("dinvrow", [1, N], f32, isOutput=False)
    dinvown_d = nc.declare_dram_parameter("dinvown", [1, ROWS], f32, isOutput=False)
    edge_d = nc.declare_dram_parameter("edge_rows", [ROWS, N], f16, isOutput=True)
    node_d = nc.declare_dram_parameter("node_rows", [2, ROWS], f32, isOutput=True)

    with tile.TileContext(nc) as tc:
        with (
            tc.tile_pool(name="singles", bufs=1) as singles,
            tc.tile_pool(name="ah", bufs=3) as ahpool,
            tc.tile_pool(name="bbf", bufs=2) as bbfpool,
            tc.tile_pool(name="rt", bufs=4) as rtpool,
            tc.tile_pool(name="eo", bufs=3) as eopool,
            tc.tile_pool(name="ps_small", bufs=2, space="PSUM") as ps_small,
            tc.tile_pool(name="ps_bb", bufs=2, space="PSUM") as ps_bb,
            tc.tile_pool(name="ps_la", bufs=2, space="PSUM") as ps_la,
            tc.tile_pool(name="ps_lb", bufs=2, space="PSUM") as ps_lb,
        ):
            # ---- input DMAs --------------------------------------------
            xT_sb = singles.tile([F_IN, N], f16)
            nc.sync.dma_start(out=xT_sb, in_=xT_d[:])
            cf16_sb = singles.tile([128, CF16], f16)
            nc.sync.dma_start(out=cf16_sb, in_=cf16_d[:])
            cf32_sb = singles.tile([128, CF32], f32)
            nc.sync.dma_start(out=cf32_sb, in_=cf32_d[:])
            dinvrow_sb = singles.tile([H, N], f32)
            _dr = dinvrow_d[:]
            nc.gpsimd.dma_start(
                out=dinvrow_sb,
                in_=bass.AP(
                    tensor=_dr.tensor,
                    offset=_dr.offset,
                    ap=[[0, H], [1, N]],
                ),
            )
            dinvown_sb = singles.tile([H, ROWS], f32)
            _do = dinvown_d[:]
            nc.gpsimd.dma_start(
                out=dinvown_sb,
                in_=bass.AP(
                    tensor=_do.tensor,
                    offset=_do.offset,
                    ap=[[0, H], [1, ROWS]],
                ),
            )
            W1_sb = cf16_sb[:, 0:H]
            Wtopb_sb = cf16_sb[0 : H + 1, H : 2 * H]
            Wbot4_sb = cf16_sb[0:H, 2 * H : 2 * H + 512].rearrange(
                "p (g n) -> p g n", g=4
            )
            Wc2sel4_sb = cf16_sb[:, 2 * H + 512 : 2 * H + 512 + 128].rearrange(
                "p (g n) -> p g n", g=4
            )
            W2_sb = cf16_sb[0:H, 2 * H + 512 + 128 : 2 * H + 512 + 130]
            b1col_sb = cf32_sb[0:H, 0:1]
            b2bc_sb = cf32_sb[:, 1:3]
            bc2col_sb = cf32_sb[:, 3:4]
            dinvcols_sb = cf32_sb[:, 4:20]
            dinvocol_sb = cf32_sb[:, 20:22]
            dinvexp_sb = cf32_sb[:, 22:278]  # [p, cb*16+k] = dinv[cb*128+p]
            b2row_sb = cf32_sb[0:2, 278:279]     # b2 as per-partition [2,1]

            # Ahat^T own columns, packed [c_local(128), (cb, i_own)]
            ahown_sb = singles.tile([128, NCB, ROWS], f8)
            nc.scalar.dma_start(out=ahown_sb, in_=Ahown_d[:])

            # ---- xw = x @ W1, packed [c_local, (cb, k)] -----------------
            ps_xw = ps_small.tile([128, NCB * H], f32, tag="small")
            for cb in range(NCB):
                nc.tensor.matmul(
                    ps_xw[:, cb * H : (cb + 1) * H],
                    xT_sb[:, cb * 128 : (cb + 1) * 128],
                    W1_sb,
                    start=True,
                    stop=True,
                )
            xw_sb = singles.tile([128, NCB * H], f16)
            nc.vector.tensor_tensor(
                out=xw_sb, in0=ps_xw, in1=dinvexp_sb, op=mybir.AluOpType.mult
            )

            # ---- conv1 on own columns -> hT_own [17, 256] ---------------
            ps_hown = ps_small.tile([H, ROWS], f32, tag="small")
            for cb in range(NCB):
                nc.tensor.matmul(
                    ps_hown,
                    xw_sb[:, cb * H : (cb + 1) * H],
                    ahown_sb[:, cb, :],
                    start=(cb == 0),
                    stop=(cb == NCB - 1),
                )
            hTown_sb = singles.tile([H + 1, ROWS], f16)
            nc.vector.memset(hTown_sb, 1.0)  # row 16 stays all-ones (bias row)
            hraw_own = singles.tile([H, ROWS], f32)
            nc.vector.tensor_tensor(
                out=hraw_own,
                in0=ps_hown,
                in1=dinvown_sb,
                op=mybir.AluOpType.mult,
            )
            nc.scalar.activation(
                out=hTown_sb[0:H, :],
                in_=hraw_own,
                func=mybir.ActivationFunctionType.Relu,
                bias=b1col_sb,
                scale=1.0,
            )

            # ---- A'_own[i_local, k] per row-block, then rearrange -------
            # A3[:, rb*16+k] = A'_rb ; A4[p, k4*8+rb*4+kg] = A3[p, rb*16+kg*4+k4]
            # T4[k4*32+i32, g32*8+rb*4+kg] = A4[g32*32+i32, k4*8+rb*4+kg]
            A3 = singles.tile([128, 32], f32)
            for rb in range(2):
                ps_ap = ps_small.tile([128, H], f32, tag="small")
                nc.tensor.matmul(
                    ps_ap,
                    hTown_sb[:, rb * 128 : (rb + 1) * 128],
                    Wtopb_sb,
                    start=True,
                    stop=True,
                )
                nc.vector.tensor_copy(A3[:, rb * H : (rb + 1) * H], ps_ap)
            A4 = singles.tile([128, 32], f32)
            a3perm = bass.AP(
                tensor=A3.tensor,
                offset=A3.offset,
                ap=[[A3.ap[0][0], 128], [1, 4], [16, 2], [4, 4]],
            )
            nc.vector.tensor_copy(A4, a3perm)
            T4_0 = singles.tile([128, 8], f32)
            T4_1 = singles.tile([128, 8], f32)
            T4_2 = singles.tile([128, 8], f32)
            T4_3 = singles.tile([128, 8], f32)
            T4g = [T4_0, T4_1, T4_2, T4_3]
            astr = A4.ap[0][0]
            for g32 in range(4):
                for k4 in range(4):
                    srcap = bass.AP(
                        tensor=A4.tensor,
                        offset=A4.offset + g32 * 32 * astr + k4 * 8,
                        ap=[[astr, 32], [1, 8]],
                    )
                    nc.gpsimd.dma_start(
                        out=T4g[g32][k4 * 32 : (k4 + 1) * 32, :],
                        in_=srcap,
                    )

            # ---- conv1 full (chunked) + B tables + edge MLP -------------
            # Superchunks of 1024 columns: TS runs [128, 1024] (one per
            # (g32, kg)), feeding two 512-wide PE matmuls.
            hT_sb = singles.tile([H, N], f16)
            for sc in range(2):
                bb4_t = bbfpool.tile([128, 4, 2 * CHUNK], f16, tag="bbf")
                for half in range(2):
                    ch = sc * 2 + half
                    cs = ch * CHUNK
                    ah_t = ahpool.tile([128, NCB, CHUNK], f8, tag="ah")
                    nc.sync.dma_start(out=ah_t, in_=AhatT_d[ch])
                    ps_h = ps_small.tile([H, CHUNK], f32, tag="small")
                    for cb in range(NCB):
                        nc.tensor.matmul(
                            ps_h,
                            xw_sb[:, cb * H : (cb + 1) * H],
                            ah_t[:, cb, :],
                            start=(cb == 0),
                            stop=(cb == NCB - 1),
                        )
                    hraw_t = eopool.tile([H, CHUNK], f32, tag="hraw")
                    nc.vector.tensor_tensor(
                        out=hraw_t,
                        in0=ps_h,
                        in1=dinvrow_sb[:, cs : cs + CHUNK],
                        op=mybir.AluOpType.mult,
                    )
                    nc.scalar.activation(
                        out=hT_sb[:, cs : cs + CHUNK],
                        in_=hraw_t,
                        func=mybir.ActivationFunctionType.Relu,
                        bias=b1col_sb,
                        scale=1.0,
                    )
                    for kg in range(4):
                        ps_b = ps_bb.tile([128, CHUNK], f32, tag="bb")
                        nc.tensor.matmul(
                            ps_b,
                            Wbot4_sb[:, kg, :],
                            hT_sb[:, cs : cs + CHUNK],
                            start=True,
                            stop=True,
                        )
                        nc.scalar.copy(
                            bb4_t[:, kg, half * CHUNK : (half + 1) * CHUNK], ps_b
                        )

                for rb in range(2):
                    ps_a0 = ps_la.tile([96, CHUNK], f32, tag="la")
                    ps_a1 = ps_la.tile([96, CHUNK], f32, tag="la")
                    ps_b20 = ps_lb.tile([32, CHUNK], f32, tag="lb")
                    ps_b21 = ps_lb.tile([32, CHUNK], f32, tag="lb")
                    ps_a = [ps_a0, ps_a1]
                    ps_b2 = [ps_b20, ps_b21]
                    for g32 in range(4):
                        for kg in range(4):
                            r_t = rtpool.tile([128, 2 * CHUNK], f16, tag="r")
                            nc.vector.tensor_scalar(
                                out=r_t,
                                in0=bb4_t[:, kg, :],
                                scalar1=T4g[g32][
                                    :,
                                    rb * 4 + kg : rb * 4 + kg + 1,
                                ],
                                scalar2=0.0,
                                op0=mybir.AluOpType.add,
                                op1=mybir.AluOpType.max,
                            )
                            for half in range(2):
                                out_ps = (
                                    ps_a[half][g32 * 32 : (g32 + 1) * 32, :]
                                    if g32 < 3
                                    else ps_b2[half]
                                )
                                nc.tensor.matmul(
                                    out_ps,
                                    Wc2sel4_sb[:, kg, :],
                                    r_t[:, half * CHUNK : (half + 1) * CHUNK],
                                    start=(kg == 0),
                                    stop=(kg == 3),
                                )
                    for half in range(2):
                        cs = (sc * 2 + half) * CHUNK
                        e_t = eopool.tile([96, CHUNK], f16, tag="e")
                        nc.scalar.activation(
                            out=e_t,
                            in_=ps_a[half],
                            func=mybir.ActivationFunctionType.Sigmoid,
                            bias=bc2col_sb[0:96, :],
                            scale=1.0,
                        )
                        e_t2 = eopool.tile([32, CHUNK], f16, tag="e2")
                        nc.scalar.activation(
                            out=e_t2,
                            in_=ps_b2[half],
                            func=mybir.ActivationFunctionType.Sigmoid,
                            bias=bc2col_sb[0:32, :],
                            scale=1.0,
                        )
                        nc.scalar.dma_start(
                            out=edge_d[rb * 128 : rb * 128 + 96, cs : cs + CHUNK],
                            in_=e_t,
                        )
                        nc.scalar.dma_start(
                            out=edge_d[rb * 128 + 96 : (rb + 1) * 128, cs : cs + CHUNK],
                            in_=e_t2,
                        )

            # ---- conv2: hw2 = h @ W2 packed [c_local, (cb, o)] ----------
            h2T_sb = singles.tile([H, N], f16)
            nc.vector.tensor_tensor(
                out=h2T_sb,
                in0=hT_sb,
                in1=dinvrow_sb,
                op=mybir.AluOpType.mult,
            )
            ps_hw2 = ps_small.tile([128, NCB * 2], f32, tag="small")
            for cb in range(NCB):
                nc.tensor.matmul(
                    ps_hw2[:, cb * 2 : (cb + 1) * 2],
                    h2T_sb[:, cb * 128 : (cb + 1) * 128],
                    W2_sb,
                    start=True,
                    stop=True,
                )
            hw2_sb = singles.tile([128, NCB * 2], f16)
            nc.vector.tensor_copy(hw2_sb, ps_hw2)

            ps_no = ps_small.tile([2, ROWS], f32, tag="small")
            for cb in range(NCB):
                nc.tensor.matmul(
                    ps_no,
                    hw2_sb[:, cb * 2 : (cb + 1) * 2],
                    ahown_sb[:, cb, :],
                    start=(cb == 0),
                    stop=(cb == NCB - 1),
                )
            no_sb = eopool.tile([2, ROWS], f32, tag="no")
            nc.vector.tensor_tensor(
                out=no_sb, in0=ps_no, in1=dinvown_sb[0:2, :], op=mybir.AluOpType.mult
            )
            nc.vector.tensor_scalar(
                out=no_sb,
                in0=no_sb,
                scalar1=b2row_sb,
                scalar2=None,
                op0=mybir.AluOpType.add,
            )
            nc.scalar.dma_start(out=node_d[:], in_=no_sb)

    nc.finalize()
    return nc


def get_program():
    if "nc" not in _PROG_CACHE:
        _PROG_CACHE["nc"] = _build_program()
    return _PROG_CACHE["nc"]


def _host_prep(x, edge_index, W1, b1, W2, b2, Wc1, bc1, Wc2, bc2):
    x = np.asarray(x, dtype=np.float32)
    ei = np.asarray(edge_index)
    src = ei[0].astype(np.int64)
    dst = ei[1].astype(np.int64)
    W1 = np.asarray(W1, np.float32)
    b1 = np.asarray(b1, np.float32)
    W2 = np.asarray(W2, np.float32)
    b2 = np.asarray(b2, np.float32)
    Wc1 = np.asarray(Wc1, np.float32)
    bc1 = np.asarray(bc1, np.float32)
    Wc2 = np.asarray(Wc2, np.float32)
    bc2 = np.asarray(bc2, np.float32)

    deg = (np.bincount(dst, minlength=N) + 1).astype(np.float32)
    dinv = 1.0 / np.sqrt(deg)
    CT = np.zeros((N, N), np.float32)
    np.add.at(CT, (src, dst), 1.0)
    idx = np.arange(N)
    CT[idx, idx] += 1.0
    assert CT.max() <= 16, "edge multiplicity too high for exact fp8 counts"
    f8np = mybir.dt.np(f8)
    CT8 = CT.astype(f8np)

    xT16 = np.ascontiguousarray(x.T).astype(np.float16)
    W1_16 = W1.astype(np.float16)
    Wtopb = np.concatenate([Wc1[:H], bc1[None, :]], axis=0).astype(np.float16)
    # Wbotrep4[f, kg, k4*32+i32] = Wc1_bot[f, kg*4+k4]
    Wbot = Wc1[H:].astype(np.float16)            # [16, 16]
    Wbotrep4 = np.zeros((H, 4, 128), np.float16)
    for kg in range(4):
        for k4 in range(4):
            Wbotrep4[:, kg, k4 * 32 : (k4 + 1) * 32] = Wbot[:, kg * 4 + k4][:, None]
    # Wc2sel4[k4*32+i32, kg, i32'] = Wc2[kg*4+k4] * (i32 == i32')
    Wc2sel4 = np.zeros((128, 4, 32), np.float16)
    eye32 = np.eye(32, dtype=np.float16)
    for kg in range(4):
        for k4 in range(4):
            Wc2sel4[k4 * 32 : (k4 + 1) * 32, kg, :] = (
                np.float16(Wc2[kg * 4 + k4, 0]) * eye32
            )
    W2_16 = W2.astype(np.float16)
    b1col = b1.reshape(H, 1).astype(np.float32)
    b2bc = np.tile(b2.reshape(1, 2), (128, 1)).astype(np.float32)
    bc2col = np.full((128, 1), bc2[0], np.float32)

    # pre-tile: AhatT_t[ch, p, cb, j] = CT8[cb*128+p, ch*CHUNK+j]
    AhatT_t = np.ascontiguousarray(
        CT8.reshape(NCB, 128, NCHUNK, CHUNK).transpose(2, 1, 0, 3)
    )
    cf16 = np.zeros((128, CF16), np.float16)
    cf16[:, 0:H] = W1_16
    cf16[0 : H + 1, H : 2 * H] = Wtopb
    cf16[0:H, 2 * H : 2 * H + 512] = Wbotrep4.reshape(H, 512)
    cf16[:, 2 * H + 512 : 2 * H + 512 + 128] = Wc2sel4.reshape(128, 128)
    cf16[0:H, 2 * H + 512 + 128 : 2 * H + 512 + 130] = W2_16
    cf32 = np.zeros((128, CF32), np.float32)
    cf32[0:H, 0:1] = b1col
    cf32[:, 1:3] = b2bc
    cf32[:, 3:4] = bc2col
    cf32[:, 4:20] = dinv.reshape(NCB, 128).T  # dinvcols[p, cb] = dinv[cb*128+p]
    cf32[:, 22:278] = np.repeat(dinv.reshape(NCB, 128).T, H, axis=1)
    cf32[0:2, 278] = b2
    shared = {
        "AhatT_t": AhatT_t,
        "xT": xT16,
        "cf16": cf16,
        "dinvrow": dinv.reshape(1, N),
    }
    in_maps = []
    for c in range(NCORES):
        m = dict(shared)
        m["AhatT_own_t"] = np.ascontiguousarray(
            CT8[:, c * ROWS : (c + 1) * ROWS]
            .reshape(NCB, 128, ROWS)
            .transpose(1, 0, 2)
        )
        dn = dinv[c * ROWS : (c + 1) * ROWS]
        m["dinvown"] = np.ascontiguousarray(dn.reshape(1, ROWS))
        cfc = cf32.copy()
        cfc[:, 20:22] = dn.reshape(2, 128).T  # dinvocol[p, rb] = dinv[own + rb*128+p]
        m["cf32"] = cfc
        in_maps.append(m)
    return in_maps


def kernel(x, edge_index, W1, b1, W2, b2, Wc1, bc1, Wc2, bc2, _res_out=None):
    in_maps = _host_prep(x, edge_index, W1, b1, W2, b2, Wc1, bc1, Wc2, bc2)
    nc = get_program()
    try:
        res = run_bass_kernel_spmd(nc, in_maps, list(range(NCORES)))
    except Exception:
        # transient device states (e.g. NRT_EXEC_UNIT_UNRECOVERABLE after a
        # wedged prior run) usually clear on retry
        import time as _t

        _t.sleep(2.0)
        res = run_bass_kernel_spmd(nc, in_maps, list(range(NCORES)))
    if _res_out is not None:
        _res_out.append(res)

    node_out = np.concatenate(
        [res.results[c]["node_rows"].T for c in range(NCORES)], axis=0
    )
    edge_out = (
        np.concatenate(
            [res.results[c]["edge_rows"] for c in range(NCORES)], axis=0
        )
        .astype(np.float32)
        .reshape(-1)
    )
    ar = np.arange(N, dtype=np.int32)
    full_edge_index = np.stack([np.repeat(ar, N), np.tile(ar, N)])
    return node_out, edge_out, full_edge_index


# revision 42
# speedup vs baseline: 1.0902x; 1.0015x over previous
"""Trainium2 Bass kernel for EnhancedGNN (2-layer GCN + all-pairs edge MLP).

Math (N=2048 nodes, F=128 in-features, H=16 hidden):
  h        = relu(Ahat @ (x @ W1) + b1)            [N, 16]
  node_out = Ahat @ (h @ W2) + b2                  [N, 2]
  E[i,j]   = sigmoid(relu([h_i, h_j] @ Wc1 + bc1) @ Wc2 + bc2)   [N, N]
  full_edge_index = all-pairs (row-major)          [2, N^2]
with Ahat = D^-1/2 (A + I) D^-1/2 built from edge_index (in-degree by dst).

Key algebra: [h_i, h_j] @ Wc1 = (h @ Wc1_top)[i] + (h @ Wc1_bot)[j], so the
N^2 x 32 pair-feature matmul collapses to an outer sum of two [N, 16] tables:
  E[i,j] = sigmoid( sum_k Wc2[k] * relu(A'[i,k] + B[j,k]) + bc2 )
  A' = h @ Wc1_top + bc1,  B = h @ Wc1_bot.

Device strategy per core (rows of E sharded, 256 rows/core):
  * conv1 replicated (every core needs the full node table for the B side),
    streamed in 4 column-chunks; all matmul operands fp16, fp32 PSUM.
  * Edge MLP on a (k4, i32) partition layout: p = k4*32 + i32.  For each
    32-row group and k-quadruple kg: DVE tensor_scalar computes
    R[p,j] = fp16(max(B[j, kg*4+k4] + A'[row(i32), kg*4+k4], 0)) in 4x mode,
    then one PE matmul with the block-diagonal stationary
    Wc2sel4[kg][p, i32'] = Wc2[kg*4+k4] * (i32==i32') accumulates the
    k-contraction straight into PSUM (M=32, legal base partitions 0/32/64;
    rows 96:128 use a second PSUM tile).  Sigmoid runs on ACT out of PSUM.
  * Per-core variation lives entirely in input data (AhatT_own slice);
    the program is identical on all 8 cores.
"""
import sys

import numpy as np

if "/opt/trn_rl_repo" not in sys.path:
    sys.path.insert(0, "/opt/trn_rl_repo")

import concourse.bass as bass
import concourse.tile as tile
from concourse import bacc, mybir
from concourse.bass_utils import run_bass_kernel_spmd

N = 2048
F_IN = 128
H = 16
NCORES = 8
ROWS = N // NCORES          # 256 rows of E per core
NCHUNK = 4
CHUNK = N // NCHUNK         # 512
NCB = N // 128              # 16 contraction blocks
f16 = mybir.dt.float16
f32 = mybir.dt.float32
f8 = mybir.dt.float8e4
CF16 = 2 * H + 512 + 130   # packed f16 consts: W1|Wtopb|Wbot4|Wc2sel4|W2
CF32 = 280                   # packed f32 consts: b1col|b2bc|bc2col

_PROG_CACHE = {}


def _build_program():
    nc = bacc.Bacc("TRN2")

    # AhatT pre-tiled host-side: [chunk, cb, 128, CHUNK] so each (chunk, cb)
    # tile is one contiguous 128KB DRAM read.
    AhatT_d = nc.declare_dram_parameter(
        "AhatT_t", [NCHUNK, 128, NCB, CHUNK], f8, isOutput=False
    )
    Ahown_d = nc.declare_dram_parameter(
        "AhatT_own_t", [128, NCB, ROWS], f8, isOutput=False
    )
    xT_d = nc.declare_dram_parameter("xT", [F_IN, N], f16, isOutput=False)
    cf16_d = nc.declare_dram_parameter("cf16", [128, CF16], f16, isOutput=False)
    cf32_d = nc.declare_dram_parameter("cf32", [128, CF32], f32, isOutput=False)
    dinvrow_d = nc.declare_dram_parameter("dinvrow", [1, N], f32, isOutput=False)
    dinvown_d = nc.declare_dram_parameter("dinvown", [1, ROWS], f32, isOutput=False)
    edge_d = nc.declare_dram_parameter("edge_rows", [ROWS, N], f16, isOutput=True)
    node_d = nc.declare_dram_parameter("node_rows", [2, ROWS], f32, isOutput=True)

    with tile.TileContext(nc) as tc:
        with (
            tc.tile_pool(name="singles", bufs=1) as singles,
            tc.tile_pool(name="ah", bufs=3) as ahpool,
            tc.tile_pool(name="bbf", bufs=2) as bbfpool,
            tc.tile_pool(name="rt", bufs=4) as rtpool,
            tc.tile_pool(name="eo", bufs=3) as eopool,
            tc.tile_pool(name="ps_small", bufs=2, space="PSUM") as ps_small,
            tc.tile_pool(name="ps_bb", bufs=2, space="PSUM") as ps_bb,
            tc.tile_pool(name="ps_la", bufs=2, space="PSUM") as ps_la,
            tc.tile_pool(name="ps_lb", bufs=2, space="PSUM") as ps_lb,
        ):
            # ---- input DMAs --------------------------------------------
            xT_sb = singles.tile([F_IN, N], f16)
            nc.sync.dma_start(out=xT_sb, in_=xT_d[:])
            cf16_sb = singles.tile([128, CF16], f16)
            nc.sync.dma_start(out=cf16_sb, in_=cf16_d[:])
            cf32_sb = singles.tile([128, CF32], f32)
            nc.sync.dma_start(out=cf32_sb, in_=cf32_d[:])
            dinvrow_sb = singles.tile([H, N], f32)
            _dr = dinvrow_d[:]
            nc.gpsimd.dma_start(
                out=dinvrow_sb,
                in_=bass.AP(
                    tensor=_dr.tensor,
                    offset=_dr.offset,
                    ap=[[0, H], [1, N]],
                ),
            )
            dinvown_sb = singles.tile([H, ROWS], f32)
            _do = dinvown_d[:]
            nc.gpsimd.dma_start(
                out=dinvown_sb,
                in_=bass.AP(
                    tensor=_do.tensor,
                    offset=_do.offset,
                    ap=[[0, H], [1, ROWS]],
                ),
            )
            W1_sb = cf16_sb[:, 0:H]
            Wtopb_sb = cf16_sb[0 : H + 1, H : 2 * H]
            Wbot4_sb = cf16_sb[0:H, 2 * H : 2 * H + 512].rearrange(
                "p (g n) -> p g n", g=4
            )
            Wc2sel4_sb = cf16_sb[:, 2 * H + 512 : 2 * H + 512 + 128].rearrange(
                "p (g n) -> p g n", g=4
            )
            W2_sb = cf16_sb[0:H, 2 * H + 512 + 128 : 2 * H + 512 + 130]
            b1col_sb = cf32_sb[0:H, 0:1]
            b2bc_sb = cf32_sb[:, 1:3]
            bc2col_sb = cf32_sb[:, 3:4]
            dinvcols_sb = cf32_sb[:, 4:20]
            dinvocol_sb = cf32_sb[:, 20:22]
            dinvexp_sb = cf32_sb[:, 22:278]  # [p, cb*16+k] = dinv[cb*128+p]
            b2row_sb = cf32_sb[0:2, 278:279]     # b2 as per-partition [2,1]

            # Ahat^T own columns, packed [c_local(128), (cb, i_own)]
            ahown_sb = singles.tile([128, NCB, ROWS], f8)
            nc.scalar.dma_start(out=ahown_sb, in_=Ahown_d[:])

            # ---- xw = x @ W1, packed [c_local, (cb, k)] -----------------
            ps_xw = ps_small.tile([128, NCB * H], f32, tag="small")
            for cb in range(NCB):
                nc.tensor.matmul(
                    ps_xw[:, cb * H : (cb + 1) * H],
                    xT_sb[:, cb * 128 : (cb + 1) * 128],
                    W1_sb,
                    start=True,
                    stop=True,
                )
            xw_sb = singles.tile([128, NCB * H], f16)
            nc.vector.tensor_tensor(
                out=xw_sb, in0=ps_xw, in1=dinvexp_sb, op=mybir.AluOpType.mult
            )

            # ---- conv1 on own columns -> hT_own [17, 256] ---------------
            ps_hown = ps_small.tile([H, ROWS], f32, tag="small")
            for cb in range(NCB):
                nc.tensor.matmul(
                    ps_hown,
                    xw_sb[:, cb * H : (cb + 1) * H],
                    ahown_sb[:, cb, :],
                    start=(cb == 0),
                    stop=(cb == NCB - 1),
                )
            hTown_sb = singles.tile([H + 1, ROWS], f16)
            nc.vector.memset(hTown_sb, 1.0)  # row 16 stays all-ones (bias row)
            hraw_own = singles.tile([H, ROWS], f32)
            nc.vector.tensor_tensor(
                out=hraw_own,
                in0=ps_hown,
                in1=dinvown_sb,
                op=mybir.AluOpType.mult,
            )
            nc.vector.tensor_scalar(
                out=hTown_sb[0:H, :],
                in0=hraw_own,
                scalar1=b1col_sb,
                scalar2=0.0,
                op0=mybir.AluOpType.add,
                op1=mybir.AluOpType.max,
            )

            # ---- A'_own[i_local, k] per row-block, then rearrange -------
            # A3[:, rb*16+k] = A'_rb ; A4[p, k4*8+rb*4+kg] = A3[p, rb*16+kg*4+k4]
            # T4[k4*32+i32, g32*8+rb*4+kg] = A4[g32*32+i32, k4*8+rb*4+kg]
            A3 = singles.tile([128, 32], f32)
            for rb in range(2):
                ps_ap = ps_small.tile([128, H], f32, tag="small")
                nc.tensor.matmul(
                    ps_ap,
                    hTown_sb[:, rb * 128 : (rb + 1) * 128],
                    Wtopb_sb,
                    start=True,
                    stop=True,
                )
                nc.vector.tensor_copy(A3[:, rb * H : (rb + 1) * H], ps_ap)
            A4 = singles.tile([128, 32], f32)
            a3perm = bass.AP(
                tensor=A3.tensor,
                offset=A3.offset,
                ap=[[A3.ap[0][0], 128], [1, 4], [16, 2], [4, 4]],
            )
            nc.vector.tensor_copy(A4, a3perm)
            T4_0 = singles.tile([128, 8], f32)
            T4_1 = singles.tile([128, 8], f32)
            T4_2 = singles.tile([128, 8], f32)
            T4_3 = singles.tile([128, 8], f32)
            T4g = [T4_0, T4_1, T4_2, T4_3]
            astr = A4.ap[0][0]
            for g32 in range(4):
                for k4 in range(4):
                    srcap = bass.AP(
                        tensor=A4.tensor,
                        offset=A4.offset + g32 * 32 * astr + k4 * 8,
                        ap=[[astr, 32], [1, 8]],
                    )
                    nc.gpsimd.dma_start(
                        out=T4g[g32][k4 * 32 : (k4 + 1) * 32, :],
                        in_=srcap,
                    )

            # ---- conv1 full (chunked) + B tables + edge MLP -------------
            # Superchunks of 1024 columns: TS runs [128, 1024] (one per
            # (g32, kg)), feeding two 512-wide PE matmuls.
            hT_sb = singles.tile([H, N], f16)
            for sc in range(2):
                bb4_t = bbfpool.tile([128, 4, 2 * CHUNK], f16, tag="bbf")
                for half in range(2):
                    ch = sc * 2 + half
                    cs = ch * CHUNK
                    ah_t = ahpool.tile([128, NCB, CHUNK], f8, tag="ah")
                    nc.sync.dma_start(out=ah_t, in_=AhatT_d[ch])
                    ps_h = ps_small.tile([H, CHUNK], f32, tag="small")
                    for cb in range(NCB):
                        nc.tensor.matmul(
                            ps_h,
                            xw_sb[:, cb * H : (cb + 1) * H],
                            ah_t[:, cb, :],
                            start=(cb == 0),
                            stop=(cb == NCB - 1),
                        )
                    hraw_t = eopool.tile([H, CHUNK], f32, tag="hraw")
                    nc.vector.tensor_tensor(
                        out=hraw_t,
                        in0=ps_h,
                        in1=dinvrow_sb[:, cs : cs + CHUNK],
                        op=mybir.AluOpType.mult,
                    )
                    nc.vector.tensor_scalar(
                        out=hT_sb[:, cs : cs + CHUNK],
                        in0=hraw_t,
                        scalar1=b1col_sb,
                        scalar2=0.0,
                        op0=mybir.AluOpType.add,
                        op1=mybir.AluOpType.max,
                    )
                    for kg in range(4):
                        ps_b = ps_bb.tile([128, CHUNK], f32, tag="bb")
                        nc.tensor.matmul(
                            ps_b,
                            Wbot4_sb[:, kg, :],
                            hT_sb[:, cs : cs + CHUNK],
                            start=True,
                            stop=True,
                        )
                        nc.scalar.copy(
                            bb4_t[:, kg, half * CHUNK : (half + 1) * CHUNK], ps_b
                        )

                for rb in range(2):
                    ps_a0 = ps_la.tile([96, CHUNK], f32, tag="la")
                    ps_a1 = ps_la.tile([96, CHUNK], f32, tag="la")
                    ps_b20 = ps_lb.tile([32, CHUNK], f32, tag="lb")
                    ps_b21 = ps_lb.tile([32, CHUNK], f32, tag="lb")
                    ps_a = [ps_a0, ps_a1]
                    ps_b2 = [ps_b20, ps_b21]
                    for g32 in range(4):
                        for kg in range(4):
                            r_t = rtpool.tile([128, 2 * CHUNK], f16, tag="r")
                            nc.vector.tensor_scalar(
                                out=r_t,
                                in0=bb4_t[:, kg, :],
                                scalar1=T4g[g32][
                                    :,
                                    rb * 4 + kg : rb * 4 + kg + 1,
                                ],
                                scalar2=0.0,
                                op0=mybir.AluOpType.add,
                                op1=mybir.AluOpType.max,
                            )
                            for half in range(2):
                                out_ps = (
                                    ps_a[half][g32 * 32 : (g32 + 1) * 32, :]
                                    if g32 < 3
                                    else ps_b2[half]
                                )
                                nc.tensor.matmul(
                                    out_ps,
                                    Wc2sel4_sb[:, kg, :],
                                    r_t[:, half * CHUNK : (half + 1) * CHUNK],
                                    start=(kg == 0),
                                    stop=(kg == 3),
                                )
                    for half in range(2):
                        cs = (sc * 2 + half) * CHUNK
                        e_t = eopool.tile([96, CHUNK], f16, tag="e")
                        nc.scalar.activation(
                            out=e_t,
                            in_=ps_a[half],
                            func=mybir.ActivationFunctionType.Sigmoid,
                            bias=bc2col_sb[0:96, :],
                            scale=1.0,
                        )
                        e_t2 = eopool.tile([32, CHUNK], f16, tag="e2")
                        nc.scalar.activation(
                            out=e_t2,
                            in_=ps_b2[half],
                            func=mybir.ActivationFunctionType.Sigmoid,
                            bias=bc2col_sb[0:32, :],
                            scale=1.0,
                        )
                        nc.scalar.dma_start(
                            out=edge_d[rb * 128 : rb * 128 + 96, cs : cs + CHUNK],
                            in_=e_t,
                        )
                        nc.scalar.dma_start(
                            out=edge_d[rb * 128 + 96 : (rb + 1) * 128, cs : cs + CHUNK],
                            in_=e_t2,
                        )

            # ---- conv2: hw2 = h @ W2 packed [c_local, (cb, o)] ----------
            h2T_sb = singles.tile([H, N], f16)
            nc.vector.tensor_tensor(
                out=h2T_sb,
                in0=hT_sb,
                in1=dinvrow_sb,
                op=mybir.AluOpType.mult,
            )
            ps_hw2 = ps_small.tile([128, NCB * 2], f32, tag="small")
            for cb in range(NCB):
                nc.tensor.matmul(
                    ps_hw2[:, cb * 2 : (cb + 1) * 2],
                    h2T_sb[:, cb * 128 : (cb + 1) * 128],
                    W2_sb,
                    start=True,
                    stop=True,
                )
            hw2_sb = singles.tile([128, NCB * 2], f16)
            nc.vector.tensor_copy(hw2_sb, ps_hw2)

            ps_no = ps_small.tile([2, ROWS], f32, tag="small")
            for cb in range(NCB):
                nc.tensor.matmul(
                    ps_no,
                    hw2_sb[:, cb * 2 : (cb + 1) * 2],
                    ahown_sb[:, cb, :],
                    start=(cb == 0),
                    stop=(cb == NCB - 1),
                )
            no_sb = eopool.tile([2, ROWS], f32, tag="no")
            nc.vector.tensor_tensor(
                out=no_sb, in0=ps_no, in1=dinvown_sb[0:2, :], op=mybir.AluOpType.mult
            )
            nc.vector.tensor_scalar(
                out=no_sb,
                in0=no_sb,
                scalar1=b2row_sb,
                scalar2=None,
                op0=mybir.AluOpType.add,
            )
            nc.scalar.dma_start(out=node_d[:], in_=no_sb)

    nc.finalize()
    return nc


def get_program():
    if "nc" not in _PROG_CACHE:
        _PROG_CACHE["nc"] = _build_program()
    return _PROG_CACHE["nc"]


def _host_prep(x, edge_index, W1, b1, W2, b2, Wc1, bc1, Wc2, bc2):
    x = np.asarray(x, dtype=np.float32)
    ei = np.asarray(edge_index)
    src = ei[0].astype(np.int64)
    dst = ei[1].astype(np.int64)
    W1 = np.asarray(W1, np.float32)
    b1 = np.asarray(b1, np.float32)
    W2 = np.asarray(W2, np.float32)
    b2 = np.asarray(b2, np.float32)
    Wc1 = np.asarray(Wc1, np.float32)
    bc1 = np.asarray(bc1, np.float32)
    Wc2 = np.asarray(Wc2, np.float32)
    bc2 = np.asarray(bc2, np.float32)

    deg = (np.bincount(dst, minlength=N) + 1).astype(np.float32)
    dinv = 1.0 / np.sqrt(deg)
    CT = np.zeros((N, N), np.float32)
    np.add.at(CT, (src, dst), 1.0)
    idx = np.arange(N)
    CT[idx, idx] += 1.0
    assert CT.max() <= 16, "edge multiplicity too high for exact fp8 counts"
    f8np = mybir.dt.np(f8)
    CT8 = CT.astype(f8np)

    xT16 = np.ascontiguousarray(x.T).astype(np.float16)
    W1_16 = W1.astype(np.float16)
    Wtopb = np.concatenate([Wc1[:H], bc1[None, :]], axis=0).astype(np.float16)
    # Wbotrep4[f, kg, k4*32+i32] = Wc1_bot[f, kg*4+k4]
    Wbot = Wc1[H:].astype(np.float16)            # [16, 16]
    Wbotrep4 = np.zeros((H, 4, 128), np.float16)
    for kg in range(4):
        for k4 in range(4):
            Wbotrep4[:, kg, k4 * 32 : (k4 + 1) * 32] = Wbot[:, kg * 4 + k4][:, None]
    # Wc2sel4[k4*32+i32, kg, i32'] = Wc2[kg*4+k4] * (i32 == i32')
    Wc2sel4 = np.zeros((128, 4, 32), np.float16)
    eye32 = np.eye(32, dtype=np.float16)
    for kg in range(4):
        for k4 in range(4):
            Wc2sel4[k4 * 32 : (k4 + 1) * 32, kg, :] = (
                np.float16(Wc2[kg * 4 + k4, 0]) * eye32
            )
    W2_16 = W2.astype(np.float16)
    b1col = b1.reshape(H, 1).astype(np.float32)
    b2bc = np.tile(b2.reshape(1, 2), (128, 1)).astype(np.float32)
    bc2col = np.full((128, 1), bc2[0], np.float32)

    # pre-tile: AhatT_t[ch, p, cb, j] = CT8[cb*128+p, ch*CHUNK+j]
    AhatT_t = np.ascontiguousarray(
        CT8.reshape(NCB, 128, NCHUNK, CHUNK).transpose(2, 1, 0, 3)
    )
    cf16 = np.zeros((128, CF16), np.float16)
    cf16[:, 0:H] = W1_16
    cf16[0 : H + 1, H : 2 * H] = Wtopb
    cf16[0:H, 2 * H : 2 * H + 512] = Wbotrep4.reshape(H, 512)
    cf16[:, 2 * H + 512 : 2 * H + 512 + 128] = Wc2sel4.reshape(128, 128)
    cf16[0:H, 2 * H + 512 + 128 : 2 * H + 512 + 130] = W2_16
    cf32 = np.zeros((128, CF32), np.float32)
    cf32[0:H, 0:1] = b1col
    cf32[:, 1:3] = b2bc
    cf32[:, 3:4] = bc2col
    cf32[:, 4:20] = dinv.reshape(NCB, 128).T  # dinvcols[p, cb] = dinv[cb*128+p]
    cf32[:, 22:278] = np.repeat(dinv.reshape(NCB, 128).T, H, axis=1)
    cf32[0:2, 278] = b2
    shared = {
        "AhatT_t": AhatT_t,
        "xT": xT16,
        "cf16": cf16,
        "dinvrow": dinv.reshape(1, N),
    }
    in_maps = []
    for c in range(NCORES):
        m = dict(shared)
        m["AhatT_own_t"] = np.ascontiguousarray(
            CT8[:, c * ROWS : (c + 1) * ROWS]
            .reshape(NCB, 128, ROWS)
            .transpose(1, 0, 2)
        )
        dn = dinv[c * ROWS : (c + 1) * ROWS]
        m["dinvown"] = np.ascontiguousarray(dn.reshape(1, ROWS))
        cfc = cf32.copy()
        cfc[:, 20:22] = dn.reshape(2, 128).T  # dinvocol[p, rb] = dinv[own + rb*128+p]
        m["cf32"] = cfc
        in_maps.append(m)
    return in_maps


def kernel(x, edge_index, W1, b1, W2, b2, Wc1, bc1, Wc2, bc2, _res_out=None):
    in_maps = _host_prep(x, edge_index, W1, b1, W2, b2, Wc1, bc1, Wc2, bc2)
    nc = get_program()
    try:
        res = run_bass_kernel_spmd(nc, in_maps, list(range(NCORES)))
    except Exception:
        # transient device states (e.g. NRT_EXEC_UNIT_UNRECOVERABLE after a
        # wedged prior run) usually clear on retry
        import time as _t

        _t.sleep(2.0)
        res = run_bass_kernel_spmd(nc, in_maps, list(range(NCORES)))
    if _res_out is not None:
        _res_out.append(res)

    node_out = np.concatenate(
        [res.results[c]["node_rows"].T for c in range(NCORES)], axis=0
    )
    edge_out = (
        np.concatenate(
            [res.results[c]["edge_rows"] for c in range(NCORES)], axis=0
        )
        .astype(np.float32)
        .reshape(-1)
    )
    ar = np.arange(N, dtype=np.int32)
    full_edge_index = np.stack([np.repeat(ar, N), np.tile(ar, N)])
    return node_out, edge_out, full_edge_index
